# revision 1
# baseline (speedup 1.0000x reference)
"""DegreeGCNLayer on 8 Trainium2 NeuronCores (Bass/Tile, SPMD).

h = (segment_sum(feature[src] * rsqrt(deg)[src], dst) * rsqrt(deg)) @ W + b

Sharding: nodes split 8 ways (9375/core); edges partitioned by dst owner on
the host so the segment-sum is core-local; feature/degree replicated to every
core (host-side replication stands in for the all-gather of remote src
features); W/b replicated.

Per-core device program:
  1. pre-scale  f~ = feature * rsqrt(degree)  -> DRAM scratch (3 chunk tables,
     32768 rows each so gather indices fit int16)
  2. dma_gather f~[src_e] per edge (SWDGE MoE gather, 1024-edge calls; edges
     sorted by (src-chunk, dst-window) on host)
  3. segment-sum on the PE: per 128-edge group build a 0/1 selection matrix
     S[j, r] = (dst_rel[j] == r) on DVE (iota compare), then
     psum_window[128 rows, 64] += S.T @ messages — exact, deterministic.
     (dma_scatter_add loses concurrent duplicate-row adds on HW, so the DMA
     CCE-add path is unusable for segment_sum.)
  4. window flush: psum -> SBUF-resident agg table (DVE add)
  5. finalize per window: scale rows by rsqrt(deg_own), PE-transpose,
     PE matmul against [W; b] (bias folded via a ones row), DMA out.
"""

import numpy as np

from concourse import bacc, bass, mybir, tile
from concourse.bass_utils import run_bass_kernel_spmd
from concourse.masks import make_identity

N_NODES = 75000
N_EDGES = 1200000
F = 64
NCORES = 8
OWN = N_NODES // NCORES            # 9375
CHUNK = 32768                      # int16-indexable gather table chunk
PAD_N = 75008                      # 9 tiles of 8192 + 1 tile of 1280
CHUNK_ROWS = (CHUNK, CHUNK, PAD_N - 2 * CHUNK)   # 32768, 32768, 9472
TILE_E = 1024                      # edges per gather call (SWDGE ring holds
                                   # 128 in-flight entries; >=2048 idxs per
                                   # call deadlocks on HW)
CHUNK_ORDER = (2, 0, 1)            # chunk 2 has the smallest pre-scale (2
                                   # tiles), so starting with it minimizes the
                                   # pipeline head before the first gather
AGG_ROWS = 9472                    # 74 windows of 128 rows (>= OWN)
N_WIN = AGG_ROWS // 128            # 74
F32 = mybir.dt.float32
I16 = mybir.dt.int16
I32 = mybir.dt.int32


def _build_nc(structure, caps, gbufs=6, sbufs=4, psbufs=4, nqueues=1,
              scratch=16384,
              ab_prescale=True, ab_gather=True, ab_sbuild=True,
              ab_matmul=True, ab_final=True):
    """Build the single SPMD Bass program.

    structure: per chunk, list of (window, n_groups) in stream order.
    caps: per-chunk padded token counts (multiples of TILE_E).
    """
    nc = bacc.Bacc("TRN2", target_bir_lowering=False, debug=False,
                   num_swdge_queues=nqueues,
                   dynamic_dma_scratch_size=scratch)

    feat = nc.declare_dram_parameter("feature", [PAD_N, F], F32, isOutput=False)
    deg = nc.declare_dram_parameter("degree", [PAD_N], F32, isOutput=False)
    tot = sum(caps)
    gidx = nc.declare_dram_parameter("gidx", [128, tot // 16], I16, isOutput=False)
    drel = nc.declare_dram_parameter("drel", [128, tot // 128], I16, isOutput=False)
    deg_own = nc.declare_dram_parameter("deg_own", [128, N_WIN], F32,
                                        isOutput=False)
    w_in = nc.declare_dram_parameter("W", [F, F], F32, isOutput=False)
    b_in = nc.declare_dram_parameter("b", [F], F32, isOutput=False)
    out = nc.declare_dram_parameter("out", [OWN, F], F32, isOutput=True)

    ftabs = [nc.dram_tensor(f"ftab{c}", [CHUNK_ROWS[c], F], F32) for c in range(3)]

    with tile.TileContext(nc) as tc:
        with (
            tc.tile_pool(name="const", bufs=1) as constp,
            tc.tile_pool(name="idxp", bufs=1) as idxp,
            tc.tile_pool(name="aggp", bufs=1) as aggp,
        ):
            # --- resident constants -------------------------------------
            # idx arrays stay SBUF-resident only when they fit; with skewed
            # edge distributions the per-core capacity (tot) can be huge, so
            # fall back to streaming per-tile slices from DRAM.
            resident = (tot // 16 + tot // 128) * 2 <= 28 * 1024
            if resident:
                gidx_sb = idxp.tile([128, tot // 16], I16, tag="gidx")
                drel_sb = idxp.tile([128, tot // 128], I16, tag="drel")
                nc.sync.dma_start(out=gidx_sb[:, :], in_=gidx[:, :])
                nc.sync.dma_start(out=drel_sb[:, :], in_=drel[:, :])

            wb = constp.tile([F, F], F32, tag="wb")
            nc.sync.dma_start(out=wb[:, :], in_=w_in[:, :])

            ident = constp.tile([128, 128], F32, tag="ident")
            make_identity(nc, ident[:, :])

            iota_mat = constp.tile([128, 128], I16, tag="iota_mat")
            nc.gpsimd.iota(iota_mat[:, :], pattern=[[1, 128]], base=0,
                           channel_multiplier=0)

            # b broadcast to all partitions via K=1 outer product with ones
            ones_row = constp.tile([1, 128], F32, tag="ones_row")
            nc.vector.memset(ones_row[:, :], 1.0)
            b_row = constp.tile([1, F], F32, tag="b_row")
            nc.sync.dma_start(out=b_row[:, :], in_=b_in[:].unsqueeze(0))
            with tc.tile_pool(name="psb", bufs=1, space="PSUM") as psbp:
                bm_ps = psbp.tile([128, F], F32, tag="bm_ps")
                nc.tensor.matmul(
                    out=bm_ps[:, :], lhsT=ones_row[:, :], rhs=b_row[:, :],
                    start=True, stop=True,
                )
                b_mat = constp.tile([128, F], F32, tag="b_mat")
                nc.vector.tensor_copy(b_mat[:, :], bm_ps[:, :])

            # rsqrt(deg_own): host ships it pre-transposed [128, 74]
            s_own = constp.tile([128, N_WIN], F32, tag="s_own")
            nc.sync.dma_start(out=s_own[:, :], in_=deg_own[:, :])
            nc.vector.reciprocal(s_own[:, :], s_own[:, :])
            nc.scalar.sqrt(s_own[:, :], s_own[:, :])

            # --- SBUF-resident agg accumulator + output buffer ----------
            agg_sb = aggp.tile([128, N_WIN, F], F32, tag="agg")
            nc.vector.memset(agg_sb[:, :, :], 0.0)
            osb_all = aggp.tile([128, N_WIN, F], F32, tag="osb_all")

            # --- per-chunk pre-scale tiles (emitted just-in-time) -------
            chunk_tiles = [
                [(t * 8192, 64) for t in range(4)],
                [(t * 8192, 64) for t in range(4, 8)],
                [(8 * 8192, 64), (73728, 10)],
            ]

            def prescale(pp, c):
                if not ab_prescale:
                    return
                for row0, G in chunk_tiles[c]:
                    nrow = 128 * G
                    ft = pp.tile([128, 64, F], F32, tag="ft")
                    dg = pp.tile([128, 64], F32, tag="dg")
                    nc.sync.dma_start(
                        out=ft[:, :G, :],
                        in_=feat[row0 : row0 + nrow, :].rearrange(
                            "(p g) f -> p g f", p=128
                        ),
                    )
                    nc.sync.dma_start(
                        out=dg[:, :G],
                        in_=deg[row0 : row0 + nrow].rearrange("(p g) -> p g", p=128),
                    )
                    nc.vector.reciprocal(dg[:, :G], dg[:, :G])
                    nc.scalar.sqrt(dg[:, :G], dg[:, :G])
                    nc.vector.tensor_tensor(
                        out=ft[:, :G, :],
                        in0=ft[:, :G, :],
                        in1=dg[:, :G].unsqueeze(2).to_broadcast([128, G, F]),
                        op=mybir.AluOpType.mult,
                    )
                    lrow = row0 - c * CHUNK
                    nc.sync.dma_start(
                        out=ftabs[c][lrow : lrow + nrow, :].rearrange(
                            "(p g) f -> p g f", p=128
                        ),
                        in_=ft[:, :G, :],
                    )

            # --- phase 2: gather + PE segment-sum -----------------------
            # last chunk (in processing order) with groups for each window
            last_chunk = {}
            for c in CHUNK_ORDER:
                for w, ngrp in structure[c]:
                    last_chunk[w] = c
            gpg = TILE_E // 128    # groups per gather tile (8)
            with (
                tc.tile_pool(name="pre", bufs=3) as pp,
                tc.tile_pool(name="gath", bufs=gbufs) as gp,
                tc.tile_pool(name="sp", bufs=sbufs) as spool,
                tc.tile_pool(name="aps", bufs=psbufs, space="PSUM") as apsp,
                tc.tile_pool(name="fin", bufs=3) as fp,
                tc.tile_pool(name="fps", bufs=2, space="PSUM") as fpsp,
            ):
                done_win = set()

                def finalize(w):
                    # h[m,:] = s[m] * (agg[m,:] @ W) + b   (row-scale commutes
                    # through the right-matmul)
                    done_win.add(w)
                    tp = fpsp.tile([F, 128], F32, tag="tp")
                    nc.tensor.transpose(
                        out=tp[:, :], in_=agg_sb[:, w, :], identity=ident[:, :]
                    )
                    acc = fp.tile([F, 128], F32, tag="acc")
                    nc.scalar.activation(
                        acc[:, :], tp[:, :], mybir.ActivationFunctionType.Copy
                    )
                    ot = fpsp.tile([128, F], F32, tag="ot")
                    nc.tensor.matmul(
                        out=ot[:, :], lhsT=acc[:, :], rhs=wb[:, :],
                        start=True, stop=True,
                    )
                    nc.vector.scalar_tensor_tensor(
                        out=osb_all[:, w, :], in0=ot[:, :],
                        scalar=s_own[:, w : w + 1], in1=b_mat[:, :],
                        op0=mybir.AluOpType.mult, op1=mybir.AluOpType.add,
                    )
                gdict = {}   # global tile index -> gather tile
                sdict = {}   # tile-pair index -> S tile ([128, 2*gpg, 128])

                def get_tile(gti, c):
                    # tile gti covers tokens [gti*TILE_E, (gti+1)*TILE_E)
                    if gti not in gdict:
                        icol = gti * TILE_E // 16
                        if resident:
                            gix = gidx_sb[:, icol : icol + TILE_E // 16]
                        else:
                            gix_t = gp.tile([128, TILE_E // 16], I16, tag="gix")
                            nc.sync.dma_start(
                                out=gix_t[:, :],
                                in_=gidx[:, icol : icol + TILE_E // 16],
                            )
                            gix = gix_t[:, :]
                        gt = gp.tile([128, gpg, F], F32, tag="gt")
                        if ab_gather:
                            nc.gpsimd.dma_gather(
                                gt[:, :, :],
                                ftabs[c][:, :],
                                gix,
                                TILE_E,
                                TILE_E,
                                F,
                                queue_num=gti % nqueues,
                            )
                        else:
                            nc.vector.memset(gt[:, 0:1, 0:1], 0.0)
                        gdict[gti] = gt
                    if gti not in sdict:
                        dcol = gti * gpg
                        if resident:
                            drl = drel_sb[:, dcol : dcol + gpg]
                        else:
                            drl_t = gp.tile([128, gpg], I16, tag="drl")
                            nc.sync.dma_start(
                                out=drl_t[:, :],
                                in_=drel[:, dcol : dcol + gpg],
                            )
                            drl = drl_t[:, :]
                        st = spool.tile([128, gpg, 128], F32, tag="st")
                        if ab_sbuild:
                            nc.vector.tensor_tensor(
                                out=st[:, :, :],
                                in0=drl.unsqueeze(2)
                                .to_broadcast([128, gpg, 128]),
                                in1=iota_mat[:, :].unsqueeze(1).to_broadcast(
                                    [128, gpg, 128]),
                                op=mybir.AluOpType.is_equal,
                            )
                        else:
                            nc.vector.memset(st[:, 0:1, 0:1], 0.0)
                        sdict[gti] = st
                    return gdict[gti], sdict[gti]

                base_tok = 0
                for c in CHUNK_ORDER:
                    prescale(pp, c)
                    g_cursor = base_tok // 128   # global group index
                    # quads of consecutive windows share one psum tile so a
                    # single DVE op flushes all of them (DVE per-op overhead
                    # dominates the tiny [128,64] adds)
                    ents = structure[c]
                    i = 0
                    while i < len(ents):
                        quad = [ents[i]]
                        while (len(quad) < 4 and i + len(quad) < len(ents)
                               and ents[i + len(quad)][0] == quad[-1][0] + 1):
                            quad.append(ents[i + len(quad)])
                        q = len(quad)
                        w0 = quad[0][0]
                        ps = apsp.tile([128, 8, F], F32, tag="ps")
                        for j, (w, ngrp) in enumerate(quad):
                            if not ab_matmul:
                                nc.vector.memset(ps[:, j, :], 0.0)
                                g_cursor += ngrp
                                continue
                            for gi in range(ngrp):
                                gcol = g_cursor + gi
                                gti = gcol // gpg
                                gt, st = get_tile(gti, c)
                                nc.tensor.matmul(
                                    out=ps[:, j, :],
                                    lhsT=st[:, gcol % gpg, :],
                                    rhs=gt[:, gcol % gpg, :],
                                    start=(gi == 0),
                                    stop=(gi == ngrp - 1),
                                )
                            g_cursor += ngrp
                        nc.vector.tensor_tensor(
                            out=agg_sb[:, w0 : w0 + q, :],
                            in0=agg_sb[:, w0 : w0 + q, :],
                            in1=ps[:, 0:q, :],
                            op=mybir.AluOpType.add,
                        )
                        if ab_final:
                            for w, _ in quad:
                                if last_chunk[w] == c:
                                    finalize(w)
                        i += q
                    base_tok += caps[c]

                # windows with no edges at all: bias-only output
                if ab_final:
                    for w in range(N_WIN):
                        if w not in done_win:
                            finalize(w)
                    # batched output writes: windows 0..72 full + 73 partial
                    nc.sync.dma_start(
                        out=out[0 : 73 * 128, :].rearrange(
                            "(w p) f -> p w f", p=128
                        ),
                        in_=osb_all[:, 0:73, :],
                    )
                    nc.sync.dma_start(
                        out=out[73 * 128 : OWN, :],
                        in_=osb_all[0 : OWN - 73 * 128, 73, :],
                    )
    nc.compile()
    return nc


def _pack_idx(vals):
    """Token i -> [i%16 + 16c, i//16] for c in 0..7 (wrap-16, replicated)."""
    cols = len(vals) // 16
    a = vals.reshape(cols, 16).T
    return np.tile(a, (8, 1)).astype(np.int16)


def _pack_tok128(vals):
    """Token i -> [i%128, i//128] layout, int16."""
    cols = len(vals) // 128
    return np.ascontiguousarray(vals.reshape(cols, 128).T.astype(np.int16))


def _prepare(feature, degree, src, dst, W, b):
    src = np.asarray(src).astype(np.int64)
    dst = np.asarray(dst).astype(np.int64)
    core = dst // OWN
    chunk = src // CHUNK
    ldst = dst - core * OWN
    win = ldst // 128

    # counts[k, c, w]
    key = (core * 3 + chunk) * N_WIN + win
    counts = np.bincount(key, minlength=NCORES * 3 * N_WIN).reshape(
        NCORES, 3, N_WIN
    )
    G = np.maximum.reduce(-(-counts // 128), axis=0)  # [3, N_WIN] group counts

    # shared program structure + per-chunk caps
    structure = []
    caps = []
    for c in range(3):
        st = [(w, int(G[c, w])) for w in range(N_WIN) if G[c, w] > 0]
        structure.append(st)
        ntok = int(G[c].sum()) * 128
        caps.append(max(TILE_E, -(-ntok // TILE_E) * TILE_E))

    # slot offsets of each (c, w) segment; chunks laid out in CHUNK_ORDER
    base = 0
    seg_off = np.zeros((3, N_WIN), np.int64)
    for c in CHUNK_ORDER:
        off = base
        for w in range(N_WIN):
            seg_off[c, w] = off
            off += int(G[c, w]) * 128
        base += caps[c]
    tot = int(base)

    feat_pad = np.zeros((PAD_N, F), np.float32)
    feat_pad[:N_NODES] = np.asarray(feature, np.float32)
    deg_pad = np.ones(PAD_N, np.float32)
    deg_pad[:N_NODES] = np.asarray(degree, np.float32)
    Wn = np.ascontiguousarray(np.asarray(W, np.float32))
    bn = np.ascontiguousarray(np.asarray(b, np.float32))
    degree_np = np.asarray(degree, np.float32)

    # per-core token placement: edges of (k,c,w) go to consecutive slots at
    # seg_off[c,w]
    order = np.argsort(key, kind="stable")
    skey = key[order]
    kstarts = np.concatenate([[0], np.cumsum(np.bincount(
        skey, minlength=NCORES * 3 * N_WIN))])

    in_maps = []
    for k in range(NCORES):
        gv = np.zeros(tot, np.int64)
        dv = np.full(tot, -1, np.int64)
        for c in range(3):
            for w in range(N_WIN):
                if G[c, w] == 0:
                    continue
                b0 = kstarts[(k * 3 + c) * N_WIN + w]
                b1 = kstarts[(k * 3 + c) * N_WIN + w + 1]
                n = b1 - b0
                o = seg_off[c, w]
                e = order[b0:b1]
                gv[o : o + n] = src[e] - c * CHUNK
                dv[o : o + n] = ldst[e] - w * 128
        do = np.ones(AGG_ROWS, np.float32)
        do[:OWN] = degree_np[k * OWN : (k + 1) * OWN]
        do = np.ascontiguousarray(do.reshape(N_WIN, 128).T)  # [128, 74]
        in_maps.append(
            {
                "feature": feat_pad,
                "degree": deg_pad,
                "gidx": _pack_idx(gv),
                "drel": _pack_tok128(dv),
                "deg_own": do,
                "W": Wn,
                "b": bn,
            }
        )
    return structure, tuple(caps), in_maps


def kernel(feature, degree, src, dst, W, b):
    structure, caps, in_maps = _prepare(feature, degree, src, dst, W, b)
    nc = _build_nc(structure, caps)
    res = run_bass_kernel_spmd(nc, in_maps, list(range(NCORES)))
    outp = np.concatenate([res.results[k]["out"] for k in range(NCORES)], axis=0)
    return outp.astype(np.float32)



# revision 6
# speedup vs baseline: 1.5038x; 1.5038x over previous
"""DegreeGCNLayer on 8 Trainium2 NeuronCores (Bass/Tile, SPMD).

h = (segment_sum(feature[src] * rsqrt(deg)[src], dst) * rsqrt(deg)) @ W + b

Sharding: nodes split 8 ways (9375/core); edges partitioned by dst owner on
the host so the segment-sum is core-local; the pre-scaled feature table is
replicated to every core (host-side replication stands in for the all-gather
of remote src features); W/b replicated.

Host prep: f~ = feature * rsqrt(degree) cast to fp16, rows padded to 128
elements (256B — the SWDGE gather descriptor minimum), split into 3 chunk
tables of <=32768 rows so gather indices fit int16.

Per-core device program:
  1. dma_gather f~[src_e] per edge (SWDGE MoE gather, 4096-edge calls; edges
     sorted by (src-chunk, dst-window) on host; 256B/edge).
  2. segment-sum on the PE: per 128-edge column build a 0/1 selection matrix
     S[j, r] = (dst_rel[j] == r) as ONE fp16 tensor_scalar(is_equal) op per
     column (per-partition scalar operand -> DVE 4x perf mode), then
     psum_win[128, 64] += S.T @ messages — exact, deterministic.
     Segments are padded to 64 tokens; a column shared by two adjacent
     windows is emitted twice with host-masked dst_rel (-1 outside the
     segment), so padding stays at 64-granularity while matmul columns
     stay 128-wide.
  3. octet flush: psum [128, <=8 windows, 64] -> SBUF agg table (DVE copy for
     the first chunk touching a window, add after).
  4. finalize per window: PE-transpose agg, cast fp16 on ACT, PE matmul
     against fp16 W, then one DVE op: out = rsqrt(deg_own) * (agg@W) + b.
  5. one partition-major output DMA [128, 74, 64]; host de-interleaves.
"""

import numpy as np

from concourse import bacc, mybir, tile
from concourse.bass_utils import run_bass_kernel_spmd
from concourse.masks import make_identity

N_NODES = 75000
N_EDGES = 1200000
F = 64
NCORES = 8
OWN = N_NODES // NCORES            # 9375
CHUNK = 32768                      # int16-indexable gather table chunk
PAD_N = 75008                      # 2*32768 + 9472
CHUNK_ROWS = (CHUNK, CHUNK, PAD_N - 2 * CHUNK)
TILE_E = 1024                      # edges per gather call; the SWDGE ring
                                   # (dynamic_dma_scratch_size/16 descriptors,
                                   # carved out of SBUF) must hold 2 calls so
                                   # desc-gen overlaps the previous transfer
GPG = TILE_E // 128                # gather columns per tile (32)
SEG_ALIGN = 64                     # segment padding granularity
AGG_ROWS = 9472                    # 74 windows of 128 rows (>= OWN)
N_WIN = AGG_ROWS // 128            # 74
F32 = mybir.dt.float32
F16 = mybir.dt.float16
I16 = mybir.dt.int16


def _build_nc(structure, caps, n_scol, first_chunk, last_chunk):
    """Build the single SPMD Bass program.

    structure: per chunk, list of (window, [(scol, gcol), ...]) in stream
    order (windows ascending, gather columns ascending).
    caps: per-chunk padded token counts (multiples of 128).
    """
    nc = bacc.Bacc("TRN2", target_bir_lowering=False, debug=False,
                   num_swdge_queues=1,
                   dynamic_dma_scratch_size=32768)

    tot = sum(caps)
    ftab = nc.declare_dram_parameter("ftab", [PAD_N, 128], F16, isOutput=False)
    gidx = nc.declare_dram_parameter("gidx", [128, tot // 16], I16, isOutput=False)
    drel = nc.declare_dram_parameter("drel", [128, n_scol], F32, isOutput=False)
    s_own_in = nc.declare_dram_parameter("s_own", [128, N_WIN], F32, isOutput=False)
    b_in = nc.declare_dram_parameter("b_mat", [128, F], F32, isOutput=False)
    w_in = nc.declare_dram_parameter("W", [F, F], F16, isOutput=False)
    iota_in = nc.declare_dram_parameter("iota", [128, 128], F16, isOutput=False)
    out = nc.declare_dram_parameter("out", [128, N_WIN, F], F32, isOutput=True)

    base_tok = [0, 0, 0]
    acc_t = 0
    for c in range(3):
        base_tok[c] = acc_t
        acc_t += caps[c]

    with tile.TileContext(nc) as tc:
        with (
            tc.tile_pool(name="const", bufs=1) as constp,
            tc.tile_pool(name="idxp", bufs=1) as idxp,
            tc.tile_pool(name="aggp", bufs=1) as aggp,
        ):
            gidx_sb = idxp.tile([128, tot // 16], I16, tag="gidx")
            drel_sb = idxp.tile([128, n_scol], F32, tag="drel")
            nc.sync.dma_start(out=gidx_sb[:, :], in_=gidx[:, :])
            nc.sync.dma_start(out=drel_sb[:, :], in_=drel[:, :])

            wb = constp.tile([F, F], F16, tag="wb")
            nc.sync.dma_start(out=wb[:, :], in_=w_in[:, :])

            s_own = constp.tile([128, N_WIN], F32, tag="s_own")
            nc.sync.dma_start(out=s_own[:, :], in_=s_own_in[:, :])

            b_mat = constp.tile([128, F], F32, tag="b_mat")
            nc.sync.dma_start(out=b_mat[:, :], in_=b_in[:, :])

            ident = constp.tile([128, 128], F32, tag="ident")
            make_identity(nc, ident[:, :])

            iota_mat = constp.tile([128, 128], F16, tag="iota_mat")
            nc.sync.dma_start(out=iota_mat[:, :], in_=iota_in[:, :])

            # --- SBUF-resident agg accumulator + output buffer ----------
            agg_sb = aggp.tile([128, N_WIN, F], F32, tag="agg")
            osb_all = aggp.tile([128, N_WIN, F], F32, tag="osb_all")
            # windows never flushed by their first chunk still need zeros
            # (finalize reads agg unconditionally)
            for w in range(N_WIN):
                if first_chunk.get(w) is None:
                    nc.vector.memset(agg_sb[:, w, :], 0.0)

            with (
                tc.tile_pool(name="gath", bufs=4) as gp,
                tc.tile_pool(name="sp", bufs=8) as spool,
                tc.tile_pool(name="aps", bufs=4, space="PSUM") as apsp,
                tc.tile_pool(name="fin", bufs=3) as fp,
                tc.tile_pool(name="fps", bufs=2, space="PSUM") as fpsp,
            ):
                done_win = set()

                def finalize(w):
                    # h[m,:] = s[m] * (agg[m,:] @ W) + b   (row-scale commutes
                    # through the right-matmul)
                    done_win.add(w)
                    tp = fpsp.tile([F, 128], F32, tag="tp")
                    nc.tensor.transpose(
                        out=tp[:, :], in_=agg_sb[:, w, :], identity=ident[:, :]
                    )
                    acc = fp.tile([F, 128], F16, tag="acc")
                    nc.scalar.activation(
                        acc[:, :], tp[:, :], mybir.ActivationFunctionType.Copy
                    )
                    ot = fpsp.tile([128, F], F32, tag="ot")
                    nc.tensor.matmul(
                        out=ot[:, :], lhsT=acc[:, :], rhs=wb[:, :],
                        start=True, stop=True,
                    )
                    nc.vector.scalar_tensor_tensor(
                        out=osb_all[:, w, :], in0=ot[:, :],
                        scalar=s_own[:, w : w + 1], in1=b_mat[:, :],
                        op0=mybir.AluOpType.mult, op1=mybir.AluOpType.add,
                    )

                gdict = {}   # (chunk, tile idx) -> gather tile

                def get_gt(c, ti):
                    if (c, ti) not in gdict:
                        tok0 = base_tok[c] + ti * TILE_E
                        ntok = min(TILE_E, base_tok[c] + caps[c] - tok0)
                        gt = gp.tile([128, GPG, 128], F16, tag="gt")
                        nc.gpsimd.dma_gather(
                            gt[:, : ntok // 128, :],
                            ftab[c * CHUNK : c * CHUNK + CHUNK_ROWS[c], :],
                            gidx_sb[:, tok0 // 16 : (tok0 + ntok) // 16],
                            ntok,
                            ntok,
                            128,
                        )
                        gdict[(c, ti)] = gt
                    return gdict[(c, ti)]

                for c in range(3):
                    ents = structure[c]
                    base_col = base_tok[c] // 128
                    i = 0
                    while i < len(ents):
                        octet = [ents[i]]
                        while (len(octet) < 8 and i + len(octet) < len(ents)
                               and ents[i + len(octet)][0]
                               == octet[-1][0] + 1):
                            octet.append(ents[i + len(octet)])
                        q = len(octet)
                        w0 = octet[0][0]
                        ps = apsp.tile([128, 8, F], F32, tag="ps")
                        for j, (w, cols) in enumerate(octet):
                            for gi, (scol, gcol) in enumerate(cols):
                                ti = (gcol - base_col) // GPG
                                gt = get_gt(c, ti)
                                lcol = gcol - base_col - ti * GPG
                                st = spool.tile([128, 128], F16, tag="st")
                                nc.vector.tensor_scalar(
                                    st[:, :], iota_mat[:, :],
                                    drel_sb[:, scol : scol + 1], None,
                                    mybir.AluOpType.is_equal,
                                )
                                nc.tensor.matmul(
                                    out=ps[:, j, :],
                                    lhsT=st[:, :],
                                    rhs=gt[:, lcol, 0:F],
                                    start=(gi == 0),
                                    stop=(gi == len(cols) - 1),
                                )
                        if first_chunk[w0] == c:
                            nc.vector.tensor_copy(
                                agg_sb[:, w0 : w0 + q, :], ps[:, 0:q, :]
                            )
                        else:
                            nc.vector.tensor_tensor(
                                out=agg_sb[:, w0 : w0 + q, :],
                                in0=agg_sb[:, w0 : w0 + q, :],
                                in1=ps[:, 0:q, :],
                                op=mybir.AluOpType.add,
                            )
                        for w, _ in octet:
                            if last_chunk[w] == c:
                                finalize(w)
                        i += q

                # windows with no edges at all: bias-only output
                for w in range(N_WIN):
                    if w not in done_win:
                        finalize(w)
                nc.sync.dma_start(out=out[:, :, :], in_=osb_all[:, :, :])
    nc.compile()
    return nc


def _pack_idx(vals):
    """Token i -> [i%16 + 16c, i//16] for c in 0..7 (wrap-16, replicated)."""
    cols = len(vals) // 16
    a = vals.reshape(cols, 16).T
    return np.tile(a, (8, 1)).astype(np.int16)


def _prepare(feature, degree, src, dst, W, b):
    src = np.asarray(src).astype(np.int64)
    dst = np.asarray(dst).astype(np.int64)
    core = dst // OWN
    chunk = src // CHUNK
    ldst = dst - core * OWN
    win = ldst // 128

    # counts[k, c, w]
    key = (core * 3 + chunk) * N_WIN + win
    counts = np.bincount(key, minlength=NCORES * 3 * N_WIN).reshape(
        NCORES, 3, N_WIN
    )
    mx = np.maximum.reduce(counts, axis=0)          # [3, N_WIN]
    seg_cap = -(-mx // SEG_ALIGN) * SEG_ALIGN       # 64-aligned capacities

    caps = []
    for c in range(3):
        ntok = int(seg_cap[c].sum())
        caps.append(max(128, -(-ntok // 128) * 128))
    tot = int(sum(caps))

    # segment offsets + shared program structure (chunk-major, window asc)
    base = 0
    seg_off = np.zeros((3, N_WIN), np.int64)
    structure = []
    scol = 0
    scol_meta = []   # (c, w, col, t0, t1) per S-column, emission order
    for c in range(3):
        off = base
        ents = []
        for w in range(N_WIN):
            if seg_cap[c, w] == 0:
                continue
            seg_off[c, w] = off
            t0, t1 = off, off + int(seg_cap[c, w])
            cols = []
            for col in range(t0 // 128, (t1 - 1) // 128 + 1):
                cols.append((scol, col))
                scol_meta.append((c, w, col, t0, t1))
                scol += 1
            ents.append((w, cols))
            off = t1
        structure.append(ents)
        base += caps[c]
    n_scol = -(-scol // 32) * 32

    first_chunk = {}
    last_chunk = {}
    for c in range(3):
        for w, _ in structure[c]:
            first_chunk.setdefault(w, c)
            last_chunk[w] = c
    for w in range(N_WIN):
        first_chunk.setdefault(w, None)

    # pre-scaled fp16 feature table, rows padded to 128 elements (256B)
    inv_sqrt = (1.0 / np.sqrt(np.asarray(degree, np.float64))).astype(np.float32)
    ftab = np.zeros((PAD_N, 128), np.float16)
    ftab[:N_NODES, :F] = (
        np.asarray(feature, np.float32) * inv_sqrt[:, None]
    ).astype(np.float16)

    Wn = np.ascontiguousarray(np.asarray(W, np.float32).astype(np.float16))
    bn = np.asarray(b, np.float32)
    b_mat = np.ascontiguousarray(np.tile(bn[None, :], (128, 1)))

    iota_host = np.ascontiguousarray(
        np.tile(np.arange(128, dtype=np.float16), (128, 1)))

    # per-core token placement: edges of (k,c,w) go to consecutive slots at
    # seg_off[c,w]
    order = np.argsort(key, kind="stable")
    kstarts = np.concatenate([[0], np.cumsum(np.bincount(
        key, minlength=NCORES * 3 * N_WIN))])

    in_maps = []
    for k in range(NCORES):
        gv = np.zeros(tot, np.int64)
        dv = np.full(tot, -1, np.int64)
        for c in range(3):
            for w in range(N_WIN):
                if seg_cap[c, w] == 0:
                    continue
                b0 = kstarts[(k * 3 + c) * N_WIN + w]
                b1 = kstarts[(k * 3 + c) * N_WIN + w + 1]
                n = b1 - b0
                o = seg_off[c, w]
                e = order[b0:b1]
                gv[o : o + n] = src[e] - c * CHUNK
                dv[o : o + n] = ldst[e] - w * 128

        # S-column dst_rel stream, masked -1 outside each segment's range
        ds = np.full((n_scol, 128), -1, np.int64)
        for s, (c, w, col, t0, t1) in enumerate(scol_meta):
            lo, hi = max(t0, col * 128), min(t1, (col + 1) * 128)
            ds[s, lo - col * 128 : hi - col * 128] = dv[lo:hi]

        so = np.ones(AGG_ROWS, np.float32)
        so[:OWN] = inv_sqrt[k * OWN : (k + 1) * OWN]
        so = np.ascontiguousarray(so.reshape(N_WIN, 128).T)  # [128, 74]
        in_maps.append(
            {
                "ftab": ftab,
                "gidx": _pack_idx(gv),
                "drel": np.ascontiguousarray(ds.T.astype(np.float32)),
                "s_own": so,
                "b_mat": b_mat,
                "W": Wn,
                "iota": iota_host,
            }
        )
    return structure, tuple(caps), n_scol, first_chunk, last_chunk, in_maps


def _unshard(res):
    """Per-core out [128, 74, 64] -> full [75000, 64] (node w*128+p)."""
    parts = []
    for k in range(NCORES):
        o = np.asarray(res[k]["out"])          # [128, N_WIN, F]
        parts.append(o.transpose(1, 0, 2).reshape(AGG_ROWS, F)[:OWN])
    return np.concatenate(parts, axis=0).astype(np.float32)


def kernel(feature, degree, src, dst, W, b):
    structure, caps, n_scol, first_chunk, last_chunk, in_maps = _prepare(
        feature, degree, src, dst, W, b
    )
    nc = _build_nc(structure, caps, n_scol, first_chunk, last_chunk)
    res = run_bass_kernel_spmd(nc, in_maps, list(range(NCORES)))
    return _unshard(res.results)


# revision 10
# speedup vs baseline: 1.5796x; 1.0504x over previous
"""DegreeGCNLayer on 8 Trainium2 NeuronCores (Bass/Tile, SPMD).

h = (segment_sum(feature[src] * rsqrt(deg)[src], dst) * rsqrt(deg)) @ W + b

Sharding: nodes split 8 ways (9375/core); edges partitioned by dst owner on
the host so the segment-sum is core-local; the pre-scaled feature table is
replicated to every core (host-side replication stands in for the all-gather
of remote src features); W/b replicated.

Host prep: f~ = feature * rsqrt(degree) cast to fp16, rows padded to 128
elements (256B — the SWDGE gather descriptor minimum), split into 3 chunk
tables of <=32768 rows so gather indices fit int16.

Per-core device program:
  1. dma_gather f~[src_e] per edge (SWDGE MoE gather, 4096-edge calls; edges
     sorted by (src-chunk, dst-window) on host; 256B/edge).
  2. segment-sum on the PE: per 128-edge column build a 0/1 selection matrix
     S[j, r] = (dst_rel[j] == r) as ONE fp16 tensor_scalar(is_equal) op per
     column (per-partition scalar operand -> DVE 4x perf mode), then
     psum_win[128, 64] += S.T @ messages — exact, deterministic.
     Segments are padded to 64 tokens; a column shared by two adjacent
     windows is emitted twice with host-masked dst_rel (-1 outside the
     segment), so padding stays at 64-granularity while matmul columns
     stay 128-wide.
  3. octet flush: psum [128, <=8 windows, 64] -> SBUF agg table (DVE copy for
     the first chunk touching a window, add after).
  4. finalize per window: PE-transpose agg, cast fp16 on ACT, PE matmul
     against fp16 W, then one DVE op: out = rsqrt(deg_own) * (agg@W) + b.
  5. one partition-major output DMA [128, 74, 64]; host de-interleaves.
"""

import numpy as np

from concourse import bacc, mybir, tile
from concourse.bass_utils import run_bass_kernel_spmd
from concourse.masks import make_identity

N_NODES = 75000
N_EDGES = 1200000
F = 64
NCORES = 8
OWN = N_NODES // NCORES            # 9375
CHUNK = 32768                      # int16-indexable gather table chunk
PAD_N = 75008                      # 2*32768 + 9472
CHUNK_ROWS = (CHUNK, CHUNK, PAD_N - 2 * CHUNK)
TILE_E = 1024                      # edges per gather call; the SWDGE ring
                                   # (dynamic_dma_scratch_size/16 descriptors,
                                   # carved out of SBUF) must hold 2 calls so
                                   # desc-gen overlaps the previous transfer
GPG = TILE_E // 128                # gather columns per tile (32)
SEG_ALIGN = 16                     # segment padding granularity
AGG_ROWS = 9472                    # 74 windows of 128 rows (>= OWN)
N_WIN = AGG_ROWS // 128            # 74
F32 = mybir.dt.float32
F16 = mybir.dt.float16
I16 = mybir.dt.int16


def _build_nc(structure, caps, n_scol, first_chunk, last_chunk):
    """Build the single SPMD Bass program.

    structure: per chunk, list of (window, [(scol, gcol), ...]) in stream
    order (windows ascending, gather columns ascending).
    caps: per-chunk padded token counts (multiples of 128).
    """
    nc = bacc.Bacc("TRN2", target_bir_lowering=False, debug=False,
                   num_swdge_queues=1,
                   dynamic_dma_scratch_size=32768)

    tot = sum(caps)
    ftab = nc.declare_dram_parameter("ftab", [PAD_N, 128], F16, isOutput=False)
    gidx = nc.declare_dram_parameter("gidx", [128, tot // 16], I16, isOutput=False)
    drel = nc.declare_dram_parameter("drel", [128, n_scol], F32, isOutput=False)
    s_own_in = nc.declare_dram_parameter("s_own", [128, N_WIN], F32, isOutput=False)
    b_in = nc.declare_dram_parameter("b_mat", [128, F], F32, isOutput=False)
    w_in = nc.declare_dram_parameter("W", [F, F], F16, isOutput=False)
    iota_in = nc.declare_dram_parameter("iota", [128, 128], F16, isOutput=False)
    out = nc.declare_dram_parameter("out", [128, N_WIN, F], F32, isOutput=True)

    base_tok = [0, 0, 0]
    acc_t = 0
    for c in range(3):
        base_tok[c] = acc_t
        acc_t += caps[c]

    with tile.TileContext(nc) as tc:
        with (
            tc.tile_pool(name="const", bufs=1) as constp,
            tc.tile_pool(name="idxp", bufs=1) as idxp,
            tc.tile_pool(name="aggp", bufs=1) as aggp,
        ):
            gidx_sb = idxp.tile([128, tot // 16], I16, tag="gidx")
            drel_sb = idxp.tile([128, n_scol], F32, tag="drel")
            nc.sync.dma_start(out=gidx_sb[:, :], in_=gidx[:, :])
            nc.sync.dma_start(out=drel_sb[:, :], in_=drel[:, :])

            wb = constp.tile([F, F], F16, tag="wb")
            nc.sync.dma_start(out=wb[:, :], in_=w_in[:, :])

            s_own = constp.tile([128, N_WIN], F32, tag="s_own")
            nc.sync.dma_start(out=s_own[:, :], in_=s_own_in[:, :])

            b_mat = constp.tile([128, F], F32, tag="b_mat")
            nc.sync.dma_start(out=b_mat[:, :], in_=b_in[:, :])

            ident = constp.tile([128, 128], F32, tag="ident")
            make_identity(nc, ident[:, :])

            iota_mat = constp.tile([128, 128], F16, tag="iota_mat")
            nc.sync.dma_start(out=iota_mat[:, :], in_=iota_in[:, :])

            # --- SBUF-resident agg accumulator + output buffer ----------
            agg_sb = aggp.tile([128, N_WIN, F], F32, tag="agg")
            osb_all = aggp.tile([128, N_WIN, F], F32, tag="osb_all")
            # windows never flushed by their first chunk still need zeros
            # (finalize reads agg unconditionally)
            for w in range(N_WIN):
                if first_chunk.get(w) is None:
                    nc.vector.memset(agg_sb[:, w, :], 0.0)

            with (
                tc.tile_pool(name="gath", bufs=16) as gp,
                tc.tile_pool(name="sp", bufs=12) as spool,
                tc.tile_pool(name="aps", bufs=4, space="PSUM") as apsp,
                tc.tile_pool(name="fin", bufs=3) as fp,
                tc.tile_pool(name="fps", bufs=2, space="PSUM") as fpsp,
            ):
                done_win = set()

                def finalize(w):
                    # h[m,:] = s[m] * (agg[m,:] @ W) + b   (row-scale commutes
                    # through the right-matmul)
                    done_win.add(w)
                    tp = fpsp.tile([F, 128], F32, tag="tp")
                    nc.tensor.transpose(
                        out=tp[:, :], in_=agg_sb[:, w, :], identity=ident[:, :]
                    )
                    acc = fp.tile([F, 128], F16, tag="acc")
                    nc.scalar.activation(
                        acc[:, :], tp[:, :], mybir.ActivationFunctionType.Copy
                    )
                    ot = fpsp.tile([128, F], F32, tag="ot")
                    nc.tensor.matmul(
                        out=ot[:, :], lhsT=acc[:, :], rhs=wb[:, :],
                        start=True, stop=True,
                    )
                    nc.vector.scalar_tensor_tensor(
                        out=osb_all[:, w, :], in0=ot[:, :],
                        scalar=s_own[:, w : w + 1], in1=b_mat[:, :],
                        op0=mybir.AluOpType.mult, op1=mybir.AluOpType.add,
                    )

                gdict = {}   # (chunk, tile idx) -> gather tile

                def get_gt(c, ti):
                    if (c, ti) not in gdict:
                        tok0 = base_tok[c] + ti * TILE_E
                        ntok = min(TILE_E, base_tok[c] + caps[c] - tok0)
                        gt = gp.tile([128, GPG, 128], F16, tag="gt")
                        nc.gpsimd.dma_gather(
                            gt[:, : ntok // 128, :],
                            ftab[c * CHUNK : c * CHUNK + CHUNK_ROWS[c], :],
                            gidx_sb[:, tok0 // 16 : (tok0 + ntok) // 16],
                            ntok,
                            ntok,
                            128,
                        )
                        gdict[(c, ti)] = gt
                    return gdict[(c, ti)]

                for c in range(3):
                    ents = structure[c]
                    base_col = base_tok[c] // 128
                    i = 0
                    while i < len(ents):
                        octet = [ents[i]]
                        while (len(octet) < 8 and i + len(octet) < len(ents)
                               and ents[i + len(octet)][0]
                               == octet[-1][0] + 1):
                            octet.append(ents[i + len(octet)])
                        q = len(octet)
                        w0 = octet[0][0]
                        ps = apsp.tile([128, 8, F], F32, tag="ps")
                        for j, (w, cols) in enumerate(octet):
                            for gi, (scol, gcol) in enumerate(cols):
                                ti = (gcol - base_col) // GPG
                                gt = get_gt(c, ti)
                                lcol = gcol - base_col - ti * GPG
                                st = spool.tile([128, 128], F16, tag="st")
                                nc.vector.tensor_scalar(
                                    st[:, :], iota_mat[:, :],
                                    drel_sb[:, scol : scol + 1], None,
                                    mybir.AluOpType.is_equal,
                                )
                                nc.tensor.matmul(
                                    out=ps[:, j, :],
                                    lhsT=st[:, :],
                                    rhs=gt[:, lcol, 0:F],
                                    start=(gi == 0),
                                    stop=(gi == len(cols) - 1),
                                )
                        if first_chunk[w0] == c:
                            nc.vector.tensor_copy(
                                agg_sb[:, w0 : w0 + q, :], ps[:, 0:q, :]
                            )
                        else:
                            nc.vector.tensor_tensor(
                                out=agg_sb[:, w0 : w0 + q, :],
                                in0=agg_sb[:, w0 : w0 + q, :],
                                in1=ps[:, 0:q, :],
                                op=mybir.AluOpType.add,
                            )
                        fin_run = []
                        for w, _ in octet:
                            if last_chunk[w] == c:
                                finalize(w)
                                fin_run.append(w)
                        if fin_run:
                            w_lo, w_hi = fin_run[0], fin_run[-1] + 1
                            nc.sync.dma_start(
                                out=out[:, w_lo:w_hi, :],
                                in_=osb_all[:, w_lo:w_hi, :],
                            )
                        i += q

                # windows with no edges at all: bias-only output
                for w in range(N_WIN):
                    if w not in done_win:
                        finalize(w)
                        nc.sync.dma_start(
                            out=out[:, w : w + 1, :],
                            in_=osb_all[:, w : w + 1, :],
                        )
    nc.compile()
    return nc


def _pack_idx(vals):
    """Token i -> [i%16 + 16c, i//16] for c in 0..7 (wrap-16, replicated)."""
    cols = len(vals) // 16
    a = vals.reshape(cols, 16).T
    return np.tile(a, (8, 1)).astype(np.int16)


def _prepare(feature, degree, src, dst, W, b):
    src = np.asarray(src).astype(np.int64)
    dst = np.asarray(dst).astype(np.int64)
    core = dst // OWN
    chunk = src // CHUNK
    ldst = dst - core * OWN
    win = ldst // 128

    # counts[k, c, w]
    key = (core * 3 + chunk) * N_WIN + win
    counts = np.bincount(key, minlength=NCORES * 3 * N_WIN).reshape(
        NCORES, 3, N_WIN
    )
    mx = np.maximum.reduce(counts, axis=0)          # [3, N_WIN]
    seg_cap = -(-mx // SEG_ALIGN) * SEG_ALIGN       # 64-aligned capacities

    caps = []
    for c in range(3):
        ntok = int(seg_cap[c].sum())
        caps.append(max(128, -(-ntok // 128) * 128))
    tot = int(sum(caps))

    # segment offsets + shared program structure (chunk-major, window asc)
    base = 0
    seg_off = np.zeros((3, N_WIN), np.int64)
    structure = []
    scol = 0
    scol_meta = []   # (c, w, col, t0, t1) per S-column, emission order
    for c in range(3):
        off = base
        ents = []
        for w in range(N_WIN):
            if seg_cap[c, w] == 0:
                continue
            seg_off[c, w] = off
            t0, t1 = off, off + int(seg_cap[c, w])
            cols = []
            for col in range(t0 // 128, (t1 - 1) // 128 + 1):
                cols.append((scol, col))
                scol_meta.append((c, w, col, t0, t1))
                scol += 1
            ents.append((w, cols))
            off = t1
        structure.append(ents)
        base += caps[c]
    n_scol = -(-scol // 32) * 32

    first_chunk = {}
    last_chunk = {}
    for c in range(3):
        for w, _ in structure[c]:
            first_chunk.setdefault(w, c)
            last_chunk[w] = c
    for w in range(N_WIN):
        first_chunk.setdefault(w, None)

    # pre-scaled fp16 feature table, rows padded to 128 elements (256B)
    inv_sqrt = (1.0 / np.sqrt(np.asarray(degree, np.float64))).astype(np.float32)
    ftab = np.zeros((PAD_N, 128), np.float16)
    ftab[:N_NODES, :F] = (
        np.asarray(feature, np.float32) * inv_sqrt[:, None]
    ).astype(np.float16)

    Wn = np.ascontiguousarray(np.asarray(W, np.float32).astype(np.float16))
    bn = np.asarray(b, np.float32)
    b_mat = np.ascontiguousarray(np.tile(bn[None, :], (128, 1)))

    iota_host = np.ascontiguousarray(
        np.tile(np.arange(128, dtype=np.float16), (128, 1)))

    # per-core token placement: edges of (k,c,w) go to consecutive slots at
    # seg_off[c,w]
    order = np.argsort(key, kind="stable")
    kstarts = np.concatenate([[0], np.cumsum(np.bincount(
        key, minlength=NCORES * 3 * N_WIN))])

    in_maps = []
    for k in range(NCORES):
        gv = np.zeros(tot, np.int64)
        dv = np.full(tot, -1, np.int64)
        for c in range(3):
            for w in range(N_WIN):
                if seg_cap[c, w] == 0:
                    continue
                b0 = kstarts[(k * 3 + c) * N_WIN + w]
                b1 = kstarts[(k * 3 + c) * N_WIN + w + 1]
                n = b1 - b0
                o = seg_off[c, w]
                e = order[b0:b1]
                gv[o : o + n] = src[e] - c * CHUNK
                dv[o : o + n] = ldst[e] - w * 128

        # S-column dst_rel stream, masked -1 outside each segment's range
        ds = np.full((n_scol, 128), -1, np.int64)
        for s, (c, w, col, t0, t1) in enumerate(scol_meta):
            lo, hi = max(t0, col * 128), min(t1, (col + 1) * 128)
            ds[s, lo - col * 128 : hi - col * 128] = dv[lo:hi]

        so = np.ones(AGG_ROWS, np.float32)
        so[:OWN] = inv_sqrt[k * OWN : (k + 1) * OWN]
        so = np.ascontiguousarray(so.reshape(N_WIN, 128).T)  # [128, 74]
        in_maps.append(
            {
                "ftab": ftab,
                "gidx": _pack_idx(gv),
                "drel": np.ascontiguousarray(ds.T.astype(np.float32)),
                "s_own": so,
                "b_mat": b_mat,
                "W": Wn,
                "iota": iota_host,
            }
        )
    return structure, tuple(caps), n_scol, first_chunk, last_chunk, in_maps


def _unshard(res):
    """Per-core out [128, 74, 64] -> full [75000, 64] (node w*128+p)."""
    parts = []
    for k in range(NCORES):
        o = np.asarray(res[k]["out"])          # [128, N_WIN, F]
        parts.append(o.transpose(1, 0, 2).reshape(AGG_ROWS, F)[:OWN])
    return np.concatenate(parts, axis=0).astype(np.float32)


def kernel(feature, degree, src, dst, W, b):
    structure, caps, n_scol, first_chunk, last_chunk, in_maps = _prepare(
        feature, degree, src, dst, W, b
    )
    nc = _build_nc(structure, caps, n_scol, first_chunk, last_chunk)
    res = run_bass_kernel_spmd(nc, in_maps, list(range(NCORES)))
    return _unshard(res.results)


# revision 11
# speedup vs baseline: 1.6691x; 1.0566x over previous
"""DegreeGCNLayer on 8 Trainium2 NeuronCores (Bass/Tile, SPMD).

h = (segment_sum(feature[src] * rsqrt(deg)[src], dst) * rsqrt(deg)) @ W + b

Sharding: nodes split 8 ways (9375/core); edges partitioned by dst owner on
the host so the segment-sum is core-local; the pre-scaled feature table is
replicated to every core (host-side replication stands in for the all-gather
of remote src features); W/b replicated.

Host prep: f~ = feature * rsqrt(degree) cast to fp16, rows padded to 128
elements (256B — the SWDGE gather descriptor minimum), split into 3 chunk
tables of <=32768 rows so gather indices fit int16.

Per-core device program:
  1. dma_gather f~[src_e] per edge (SWDGE MoE gather, 4096-edge calls; edges
     sorted by (src-chunk, dst-window) on host; 256B/edge).
  2. segment-sum on the PE: per 128-edge column build a 0/1 selection matrix
     S[j, r] = (dst_rel[j] == r) as ONE fp16 tensor_scalar(is_equal) op per
     column (per-partition scalar operand -> DVE 4x perf mode), then
     psum_win[128, 64] += S.T @ messages — exact, deterministic.
     Segments are padded to 64 tokens; a column shared by two adjacent
     windows is emitted twice with host-masked dst_rel (-1 outside the
     segment), so padding stays at 64-granularity while matmul columns
     stay 128-wide.
  3. octet flush: psum [128, <=8 windows, 64] -> SBUF agg table (DVE copy for
     the first chunk touching a window, add after).
  4. finalize per window: PE-transpose agg, cast fp16 on ACT, PE matmul
     against fp16 W, then one DVE op: out = rsqrt(deg_own) * (agg@W) + b.
  5. one partition-major output DMA [128, 74, 64]; host de-interleaves.
"""

import numpy as np

from concourse import bacc, mybir, tile
from concourse.bass_utils import run_bass_kernel_spmd
from concourse.masks import make_identity

N_NODES = 75000
N_EDGES = 1200000
F = 64
NCORES = 8
OWN = N_NODES // NCORES            # 9375
CHUNK = 32768                      # int16-indexable gather table chunk
PAD_N = 75008                      # 2*32768 + 9472
CHUNK_ROWS = (CHUNK, CHUNK, PAD_N - 2 * CHUNK)
TILE_E = 1024                      # edges per gather call; the SWDGE ring
                                   # (dynamic_dma_scratch_size/16 descriptors,
                                   # carved out of SBUF) must hold 2 calls so
                                   # desc-gen overlaps the previous transfer
GPG = TILE_E // 128                # gather columns per tile (32)
CH_ORDER = (2, 0, 1)               # chunk 2 first: window finalizes (all in
                                   # the last chunk's sweep) hide under the
                                   # big chunk-1 gather stream
SEG_ALIGN = 16                     # segment padding granularity
AGG_ROWS = 9472                    # 74 windows of 128 rows (>= OWN)
N_WIN = AGG_ROWS // 128            # 74
F32 = mybir.dt.float32
F16 = mybir.dt.float16
I16 = mybir.dt.int16


def _build_nc(structure, caps, n_scol, first_chunk, last_chunk):
    """Build the single SPMD Bass program.

    structure: per chunk, list of (window, [(scol, gcol), ...]) in stream
    order (windows ascending, gather columns ascending).
    caps: per-chunk padded token counts (multiples of 128).
    """
    nc = bacc.Bacc("TRN2", target_bir_lowering=False, debug=False,
                   num_swdge_queues=1,
                   dynamic_dma_scratch_size=32768)

    tot = sum(caps)
    ftab = nc.declare_dram_parameter("ftab", [PAD_N, 128], F16, isOutput=False)
    gidx = nc.declare_dram_parameter("gidx", [128, tot // 16], I16, isOutput=False)
    drel = nc.declare_dram_parameter("drel", [128, n_scol], F32, isOutput=False)
    s_own_in = nc.declare_dram_parameter("s_own", [128, N_WIN], F32, isOutput=False)
    b_in = nc.declare_dram_parameter("b_mat", [128, F], F32, isOutput=False)
    w_in = nc.declare_dram_parameter("W", [F, F], F16, isOutput=False)
    iota_in = nc.declare_dram_parameter("iota", [128, 128], F16, isOutput=False)
    out = nc.declare_dram_parameter("out", [128, N_WIN, F], F32, isOutput=True)

    base_tok = [0, 0, 0]
    acc_t = 0
    for c in CH_ORDER:
        base_tok[c] = acc_t
        acc_t += caps[c]

    with tile.TileContext(nc) as tc:
        with (
            tc.tile_pool(name="const", bufs=1) as constp,
            tc.tile_pool(name="idxp", bufs=1) as idxp,
            tc.tile_pool(name="aggp", bufs=1) as aggp,
        ):
            gidx_sb = idxp.tile([128, tot // 16], I16, tag="gidx")
            drel_sb = idxp.tile([128, n_scol], F32, tag="drel")
            nc.sync.dma_start(out=gidx_sb[:, :], in_=gidx[:, :])
            nc.sync.dma_start(out=drel_sb[:, :], in_=drel[:, :])

            wb = constp.tile([F, F], F16, tag="wb")
            nc.sync.dma_start(out=wb[:, :], in_=w_in[:, :])

            s_own = constp.tile([128, N_WIN], F32, tag="s_own")
            nc.sync.dma_start(out=s_own[:, :], in_=s_own_in[:, :])

            b_mat = constp.tile([128, F], F32, tag="b_mat")
            nc.sync.dma_start(out=b_mat[:, :], in_=b_in[:, :])

            ident = constp.tile([128, 128], F32, tag="ident")
            make_identity(nc, ident[:, :])

            iota_mat = constp.tile([128, 128], F16, tag="iota_mat")
            nc.sync.dma_start(out=iota_mat[:, :], in_=iota_in[:, :])

            # --- SBUF-resident agg accumulator + output buffer ----------
            agg_sb = aggp.tile([128, N_WIN, F], F32, tag="agg")
            osb_all = aggp.tile([128, N_WIN, F], F32, tag="osb_all")
            # windows never flushed by their first chunk still need zeros
            # (finalize reads agg unconditionally)
            for w in range(N_WIN):
                if first_chunk.get(w) is None:
                    nc.vector.memset(agg_sb[:, w, :], 0.0)

            with (
                tc.tile_pool(name="gath", bufs=16) as gp,
                tc.tile_pool(name="sp", bufs=12) as spool,
                tc.tile_pool(name="aps", bufs=4, space="PSUM") as apsp,
                tc.tile_pool(name="fin", bufs=3) as fp,
                tc.tile_pool(name="fps", bufs=2, space="PSUM") as fpsp,
            ):
                done_win = set()

                def finalize(w):
                    # h[m,:] = s[m] * (agg[m,:] @ W) + b   (row-scale commutes
                    # through the right-matmul)
                    done_win.add(w)
                    tp = fpsp.tile([F, 128], F32, tag="tp")
                    nc.tensor.transpose(
                        out=tp[:, :], in_=agg_sb[:, w, :], identity=ident[:, :]
                    )
                    acc = fp.tile([F, 128], F16, tag="acc")
                    nc.scalar.activation(
                        acc[:, :], tp[:, :], mybir.ActivationFunctionType.Copy
                    )
                    ot = fpsp.tile([128, F], F32, tag="ot")
                    nc.tensor.matmul(
                        out=ot[:, :], lhsT=acc[:, :], rhs=wb[:, :],
                        start=True, stop=True,
                    )
                    nc.vector.scalar_tensor_tensor(
                        out=osb_all[:, w, :], in0=ot[:, :],
                        scalar=s_own[:, w : w + 1], in1=b_mat[:, :],
                        op0=mybir.AluOpType.mult, op1=mybir.AluOpType.add,
                    )

                gdict = {}   # (chunk, tile idx) -> gather tile

                def get_gt(c, ti):
                    if (c, ti) not in gdict:
                        tok0 = base_tok[c] + ti * TILE_E
                        ntok = min(TILE_E, base_tok[c] + caps[c] - tok0)
                        gt = gp.tile([128, GPG, 128], F16, tag="gt")
                        nc.gpsimd.dma_gather(
                            gt[:, : ntok // 128, :],
                            ftab[c * CHUNK : c * CHUNK + CHUNK_ROWS[c], :],
                            gidx_sb[:, tok0 // 16 : (tok0 + ntok) // 16],
                            ntok,
                            ntok,
                            128,
                        )
                        gdict[(c, ti)] = gt
                    return gdict[(c, ti)]

                for c in CH_ORDER:
                    ents = structure[c]
                    base_col = base_tok[c] // 128
                    i = 0
                    while i < len(ents):
                        octet = [ents[i]]
                        while (len(octet) < 8 and i + len(octet) < len(ents)
                               and ents[i + len(octet)][0]
                               == octet[-1][0] + 1):
                            octet.append(ents[i + len(octet)])
                        q = len(octet)
                        w0 = octet[0][0]
                        ps = apsp.tile([128, 8, F], F32, tag="ps")
                        for j, (w, cols) in enumerate(octet):
                            for gi, (scol, gcol) in enumerate(cols):
                                ti = (gcol - base_col) // GPG
                                gt = get_gt(c, ti)
                                lcol = gcol - base_col - ti * GPG
                                st = spool.tile([128, 128], F16, tag="st")
                                nc.vector.tensor_scalar(
                                    st[:, :], iota_mat[:, :],
                                    drel_sb[:, scol : scol + 1], None,
                                    mybir.AluOpType.is_equal,
                                )
                                nc.tensor.matmul(
                                    out=ps[:, j, :],
                                    lhsT=st[:, :],
                                    rhs=gt[:, lcol, 0:F],
                                    start=(gi == 0),
                                    stop=(gi == len(cols) - 1),
                                )
                        if first_chunk[w0] == c:
                            nc.vector.tensor_copy(
                                agg_sb[:, w0 : w0 + q, :], ps[:, 0:q, :]
                            )
                        else:
                            nc.vector.tensor_tensor(
                                out=agg_sb[:, w0 : w0 + q, :],
                                in0=agg_sb[:, w0 : w0 + q, :],
                                in1=ps[:, 0:q, :],
                                op=mybir.AluOpType.add,
                            )
                        fin_run = []
                        for w, _ in octet:
                            if last_chunk[w] == c:
                                finalize(w)
                                fin_run.append(w)
                        if fin_run:
                            w_lo, w_hi = fin_run[0], fin_run[-1] + 1
                            nc.sync.dma_start(
                                out=out[:, w_lo:w_hi, :],
                                in_=osb_all[:, w_lo:w_hi, :],
                            )
                        i += q

                # windows with no edges at all: bias-only output
                for w in range(N_WIN):
                    if w not in done_win:
                        finalize(w)
                        nc.sync.dma_start(
                            out=out[:, w : w + 1, :],
                            in_=osb_all[:, w : w + 1, :],
                        )
    nc.compile()
    return nc


def _pack_idx(vals):
    """Token i -> [i%16 + 16c, i//16] for c in 0..7 (wrap-16, replicated)."""
    cols = len(vals) // 16
    a = vals.reshape(cols, 16).T
    return np.tile(a, (8, 1)).astype(np.int16)


def _prepare(feature, degree, src, dst, W, b):
    src = np.asarray(src).astype(np.int64)
    dst = np.asarray(dst).astype(np.int64)
    core = dst // OWN
    chunk = src // CHUNK
    ldst = dst - core * OWN
    win = ldst // 128

    # counts[k, c, w]
    key = (core * 3 + chunk) * N_WIN + win
    counts = np.bincount(key, minlength=NCORES * 3 * N_WIN).reshape(
        NCORES, 3, N_WIN
    )
    mx = np.maximum.reduce(counts, axis=0)          # [3, N_WIN]
    seg_cap = -(-mx // SEG_ALIGN) * SEG_ALIGN       # 64-aligned capacities

    caps = []
    for c in range(3):
        ntok = int(seg_cap[c].sum())
        caps.append(max(128, -(-ntok // 128) * 128))
    tot = int(sum(caps))

    # segment offsets + shared program structure (chunk-major, window asc)
    base = 0
    seg_off = np.zeros((3, N_WIN), np.int64)
    structure = [None, None, None]
    scol = 0
    scol_meta = []   # (c, w, col, t0, t1) per S-column, emission order
    for c in CH_ORDER:
        off = base
        ents = []
        for w in range(N_WIN):
            if seg_cap[c, w] == 0:
                continue
            seg_off[c, w] = off
            t0, t1 = off, off + int(seg_cap[c, w])
            cols = []
            for col in range(t0 // 128, (t1 - 1) // 128 + 1):
                cols.append((scol, col))
                scol_meta.append((c, w, col, t0, t1))
                scol += 1
            ents.append((w, cols))
            off = t1
        structure[c] = ents
        base += caps[c]
    n_scol = -(-scol // 32) * 32

    first_chunk = {}
    last_chunk = {}
    for c in CH_ORDER:
        for w, _ in structure[c]:
            first_chunk.setdefault(w, c)
            last_chunk[w] = c
    for w in range(N_WIN):
        first_chunk.setdefault(w, None)

    # pre-scaled fp16 feature table, rows padded to 128 elements (256B)
    inv_sqrt = (1.0 / np.sqrt(np.asarray(degree, np.float64))).astype(np.float32)
    ftab = np.zeros((PAD_N, 128), np.float16)
    ftab[:N_NODES, :F] = (
        np.asarray(feature, np.float32) * inv_sqrt[:, None]
    ).astype(np.float16)

    Wn = np.ascontiguousarray(np.asarray(W, np.float32).astype(np.float16))
    bn = np.asarray(b, np.float32)
    b_mat = np.ascontiguousarray(np.tile(bn[None, :], (128, 1)))

    iota_host = np.ascontiguousarray(
        np.tile(np.arange(128, dtype=np.float16), (128, 1)))

    # per-core token placement: edges of (k,c,w) go to consecutive slots at
    # seg_off[c,w]
    order = np.argsort(key, kind="stable")
    kstarts = np.concatenate([[0], np.cumsum(np.bincount(
        key, minlength=NCORES * 3 * N_WIN))])

    in_maps = []
    for k in range(NCORES):
        gv = np.zeros(tot, np.int64)
        dv = np.full(tot, -1, np.int64)
        for c in range(3):
            for w in range(N_WIN):
                if seg_cap[c, w] == 0:
                    continue
                b0 = kstarts[(k * 3 + c) * N_WIN + w]
                b1 = kstarts[(k * 3 + c) * N_WIN + w + 1]
                n = b1 - b0
                o = seg_off[c, w]
                e = order[b0:b1]
                gv[o : o + n] = src[e] - c * CHUNK
                dv[o : o + n] = ldst[e] - w * 128

        # S-column dst_rel stream, masked -1 outside each segment's range
        ds = np.full((n_scol, 128), -1, np.int64)
        for s, (c, w, col, t0, t1) in enumerate(scol_meta):
            lo, hi = max(t0, col * 128), min(t1, (col + 1) * 128)
            ds[s, lo - col * 128 : hi - col * 128] = dv[lo:hi]

        so = np.ones(AGG_ROWS, np.float32)
        so[:OWN] = inv_sqrt[k * OWN : (k + 1) * OWN]
        so = np.ascontiguousarray(so.reshape(N_WIN, 128).T)  # [128, 74]
        in_maps.append(
            {
                "ftab": ftab,
                "gidx": _pack_idx(gv),
                "drel": np.ascontiguousarray(ds.T.astype(np.float32)),
                "s_own": so,
                "b_mat": b_mat,
                "W": Wn,
                "iota": iota_host,
            }
        )
    return structure, tuple(caps), n_scol, first_chunk, last_chunk, in_maps


def _unshard(res):
    """Per-core out [128, 74, 64] -> full [75000, 64] (node w*128+p)."""
    parts = []
    for k in range(NCORES):
        o = np.asarray(res[k]["out"])          # [128, N_WIN, F]
        parts.append(o.transpose(1, 0, 2).reshape(AGG_ROWS, F)[:OWN])
    return np.concatenate(parts, axis=0).astype(np.float32)


def kernel(feature, degree, src, dst, W, b):
    structure, caps, n_scol, first_chunk, last_chunk, in_maps = _prepare(
        feature, degree, src, dst, W, b
    )
    nc = _build_nc(structure, caps, n_scol, first_chunk, last_chunk)
    res = run_bass_kernel_spmd(nc, in_maps, list(range(NCORES)))
    return _unshard(res.results)


# revision 12
# speedup vs baseline: 1.7154x; 1.0278x over previous
"""DegreeGCNLayer on 8 Trainium2 NeuronCores (Bass/Tile, SPMD).

h = (segment_sum(feature[src] * rsqrt(deg)[src], dst) * rsqrt(deg)) @ W + b

Sharding: nodes split 8 ways (9375/core); edges partitioned by dst owner on
the host so the segment-sum is core-local; the pre-scaled feature table is
replicated to every core (host-side replication stands in for the all-gather
of remote src features); W/b replicated.

Host prep: f~ = feature * rsqrt(degree) cast to fp16, rows padded to 128
elements (256B — the SWDGE gather descriptor minimum), split into 3 chunk
tables of <=32768 rows so gather indices fit int16.

Per-core device program:
  1. dma_gather f~[src_e] per edge (SWDGE MoE gather, 4096-edge calls; edges
     sorted by (src-chunk, dst-window) on host; 256B/edge).
  2. segment-sum on the PE: per 128-edge column build a 0/1 selection matrix
     S[j, r] = (dst_rel[j] == r) as ONE fp16 tensor_scalar(is_equal) op per
     column (per-partition scalar operand -> DVE 4x perf mode), then
     psum_win[128, 64] += S.T @ messages — exact, deterministic.
     Segments are padded to 64 tokens; a column shared by two adjacent
     windows is emitted twice with host-masked dst_rel (-1 outside the
     segment), so padding stays at 64-granularity while matmul columns
     stay 128-wide.
  3. octet flush: psum [128, <=8 windows, 64] -> SBUF agg table (DVE copy for
     the first chunk touching a window, add after).
  4. finalize per window: PE-transpose agg, cast fp16 on ACT, PE matmul
     against fp16 W, then one DVE op: out = rsqrt(deg_own) * (agg@W) + b.
  5. one partition-major output DMA [128, 74, 64]; host de-interleaves.
"""

import numpy as np

from concourse import bacc, mybir, tile
from concourse.bass_utils import run_bass_kernel_spmd
from concourse.masks import make_identity

N_NODES = 75000
N_EDGES = 1200000
F = 64
NCORES = 8
OWN = N_NODES // NCORES            # 9375
CHUNK = 32768                      # int16-indexable gather table chunk
PAD_N = 75008                      # 2*32768 + 9472
CHUNK_ROWS = (CHUNK, CHUNK, PAD_N - 2 * CHUNK)
TILE_E = 1024                      # edges per gather call; the SWDGE ring
                                   # (dynamic_dma_scratch_size/16 descriptors,
                                   # carved out of SBUF) must hold 2 calls so
                                   # desc-gen overlaps the previous transfer
GPG = TILE_E // 128                # gather columns per tile (32)
CH_ORDER = (2, 0, 1)               # chunk 2 first: window finalizes (all in
                                   # the last chunk's sweep) hide under the
                                   # big chunk-1 gather stream
SEG_ALIGN = 16                     # segment padding granularity
AGG_ROWS = 9472                    # 74 windows of 128 rows (>= OWN)
N_WIN = AGG_ROWS // 128            # 74
F32 = mybir.dt.float32
F16 = mybir.dt.float16
I16 = mybir.dt.int16


def _build_nc(structure, caps, n_scol, first_chunk, last_chunk):
    """Build the single SPMD Bass program.

    structure: per chunk, list of (window, [(scol, gcol), ...]) in stream
    order (windows ascending, gather columns ascending).
    caps: per-chunk padded token counts (multiples of 128).
    """
    nc = bacc.Bacc("TRN2", target_bir_lowering=False, debug=False,
                   num_swdge_queues=1,
                   dynamic_dma_scratch_size=32768)

    tot = sum(caps)
    ftab = nc.declare_dram_parameter("ftab", [PAD_N, 128], F16, isOutput=False)
    gidx = nc.declare_dram_parameter("gidx", [128, tot // 16], I16, isOutput=False)
    drel = nc.declare_dram_parameter("drel", [128, n_scol], F32, isOutput=False)
    s_own_in = nc.declare_dram_parameter("s_own", [128, N_WIN], F32, isOutput=False)
    b_in = nc.declare_dram_parameter("b_mat", [128, F], F32, isOutput=False)
    w_in = nc.declare_dram_parameter("W", [F, F], F16, isOutput=False)
    iota_in = nc.declare_dram_parameter("iota", [128, 128], F16, isOutput=False)
    out = nc.declare_dram_parameter("out", [128, N_WIN, F], F32, isOutput=True)

    base_tok = [0, 0, 0]
    acc_t = 0
    for c in CH_ORDER:
        base_tok[c] = acc_t
        acc_t += caps[c]

    with tile.TileContext(nc) as tc:
        with (
            tc.tile_pool(name="const", bufs=1) as constp,
            tc.tile_pool(name="idxp", bufs=1) as idxp,
            tc.tile_pool(name="aggp", bufs=1) as aggp,
        ):
            gidx_sb = idxp.tile([128, tot // 16], I16, tag="gidx")
            drel_sb = idxp.tile([128, n_scol], F32, tag="drel")
            nc.sync.dma_start(out=gidx_sb[:, :], in_=gidx[:, :])
            nc.sync.dma_start(out=drel_sb[:, :], in_=drel[:, :])

            wb = constp.tile([F, F], F16, tag="wb")
            nc.sync.dma_start(out=wb[:, :], in_=w_in[:, :])

            s_own = constp.tile([128, N_WIN], F32, tag="s_own")
            nc.sync.dma_start(out=s_own[:, :], in_=s_own_in[:, :])

            b_mat = constp.tile([128, F], F32, tag="b_mat")
            nc.sync.dma_start(out=b_mat[:, :], in_=b_in[:, :])

            ident = constp.tile([128, 128], F32, tag="ident")
            make_identity(nc, ident[:, :])

            iota_mat = constp.tile([128, 128], F16, tag="iota_mat")
            nc.sync.dma_start(out=iota_mat[:, :], in_=iota_in[:, :])

            # --- SBUF-resident agg accumulator + output buffer ----------
            agg_sb = aggp.tile([128, N_WIN, F], F32, tag="agg")
            osb_all = aggp.tile([128, N_WIN, F], F32, tag="osb_all")
            # windows never flushed by their first chunk still need zeros
            # (finalize reads agg unconditionally)
            for w in range(N_WIN):
                if first_chunk.get(w) is None:
                    nc.vector.memset(agg_sb[:, w, :], 0.0)

            with (
                tc.tile_pool(name="gath", bufs=16) as gp,
                tc.tile_pool(name="sp", bufs=12) as spool,
                tc.tile_pool(name="aps", bufs=4, space="PSUM") as apsp,
                tc.tile_pool(name="fin", bufs=3) as fp,
                tc.tile_pool(name="fps", bufs=2, space="PSUM") as fpsp,
            ):
                done_win = set()

                def finalize(w):
                    # h[m,:] = s[m] * (agg[m,:] @ W) + b   (row-scale commutes
                    # through the right-matmul)
                    done_win.add(w)
                    tp = fpsp.tile([F, 128], F32, tag="tp")
                    nc.tensor.transpose(
                        out=tp[:, :], in_=agg_sb[:, w, :], identity=ident[:, :]
                    )
                    acc = fp.tile([F, 128], F16, tag="acc")
                    nc.scalar.activation(
                        acc[:, :], tp[:, :], mybir.ActivationFunctionType.Copy
                    )
                    ot = fpsp.tile([128, F], F32, tag="ot")
                    nc.tensor.matmul(
                        out=ot[:, :], lhsT=acc[:, :], rhs=wb[:, :],
                        start=True, stop=True,
                    )
                    nc.vector.scalar_tensor_tensor(
                        out=osb_all[:, w, :], in0=ot[:, :],
                        scalar=s_own[:, w : w + 1], in1=b_mat[:, :],
                        op0=mybir.AluOpType.mult, op1=mybir.AluOpType.add,
                    )

                gdict = {}   # (chunk, tile idx) -> gather tile

                def get_gt(c, ti):
                    if (c, ti) not in gdict:
                        tok0 = base_tok[c] + ti * TILE_E
                        ntok = min(TILE_E, base_tok[c] + caps[c] - tok0)
                        gt = gp.tile([128, GPG, 128], F16, tag="gt")
                        nc.gpsimd.dma_gather(
                            gt[:, : ntok // 128, :],
                            ftab[c * CHUNK : c * CHUNK + CHUNK_ROWS[c], :],
                            gidx_sb[:, tok0 // 16 : (tok0 + ntok) // 16],
                            ntok,
                            ntok,
                            128,
                        )
                        gdict[(c, ti)] = gt
                    return gdict[(c, ti)]

                for c in CH_ORDER:
                    ents = structure[c]
                    base_col = base_tok[c] // 128
                    i = 0
                    while i < len(ents):
                        octet = [ents[i]]
                        while (len(octet) < 8 and i + len(octet) < len(ents)
                               and ents[i + len(octet)][0]
                               == octet[-1][0] + 1):
                            octet.append(ents[i + len(octet)])
                        q = len(octet)
                        w0 = octet[0][0]
                        ps = apsp.tile([128, 8, F], F32, tag="ps")
                        for j, (w, cols) in enumerate(octet):
                            for gi, (scol, gcol) in enumerate(cols):
                                ti = (gcol - base_col) // GPG
                                gt = get_gt(c, ti)
                                lcol = gcol - base_col - ti * GPG
                                st = spool.tile([128, 128], F16, tag="st")
                                nc.vector.tensor_scalar(
                                    st[:, :], iota_mat[:, :],
                                    drel_sb[:, scol : scol + 1], None,
                                    mybir.AluOpType.is_equal,
                                )
                                nc.tensor.matmul(
                                    out=ps[:, j, :],
                                    lhsT=st[:, :],
                                    rhs=gt[:, lcol, 0:F],
                                    start=(gi == 0),
                                    stop=(gi == len(cols) - 1),
                                )
                        if first_chunk[w0] == c:
                            nc.vector.tensor_copy(
                                agg_sb[:, w0 : w0 + q, :], ps[:, 0:q, :]
                            )
                        else:
                            nc.vector.tensor_tensor(
                                out=agg_sb[:, w0 : w0 + q, :],
                                in0=agg_sb[:, w0 : w0 + q, :],
                                in1=ps[:, 0:q, :],
                                op=mybir.AluOpType.add,
                            )
                        fin_run = []
                        for w, _ in octet:
                            if last_chunk[w] == c:
                                finalize(w)
                                fin_run.append(w)
                        if fin_run:
                            w_lo, w_hi = fin_run[0], fin_run[-1] + 1
                            nc.sync.dma_start(
                                out=out[:, w_lo:w_hi, :],
                                in_=osb_all[:, w_lo:w_hi, :],
                            )
                        i += q

                # windows with no edges at all: bias-only output
                for w in range(N_WIN):
                    if w not in done_win:
                        finalize(w)
                        nc.sync.dma_start(
                            out=out[:, w : w + 1, :],
                            in_=osb_all[:, w : w + 1, :],
                        )
    nc.compile()
    return nc


def _pack_idx(vals):
    """Token i -> [i%16 + 16c, i//16] for c in 0..7 (wrap-16, replicated)."""
    cols = len(vals) // 16
    a = vals.reshape(cols, 16).T
    return np.tile(a, (8, 1)).astype(np.int16)


def _window_perms(counts):
    """Per-core window->slot permutation equalizing per-slot loads across
    cores (shrinks the shared program's max-over-cores segment capacities).

    Returns perm[k, s] = physical window of core k living in slot s.
    """
    tot_kw = counts.sum(axis=1)
    perm = np.argsort(-tot_kw, axis=1)
    cnt_p = np.take_along_axis(counts, perm[:, None, :], axis=2)
    try:
        from scipy.optimize import linear_sum_assignment
    except Exception:
        return perm
    for _ in range(5):
        for k in range(NCORES):
            others = np.maximum.reduce(np.delete(cnt_p, k, axis=0), axis=0)
            ck = cnt_p[k]
            m = np.maximum(others[:, :, None], ck[:, None, :])
            cost = (-(-m // SEG_ALIGN) * SEG_ALIGN).sum(axis=0)
            _, cidx = linear_sum_assignment(cost)
            cnt_p[k] = ck[:, cidx]
            perm[k] = perm[k][cidx]
    return perm


def _prepare(feature, degree, src, dst, W, b):
    src = np.asarray(src).astype(np.int64)
    dst = np.asarray(dst).astype(np.int64)
    core = dst // OWN
    chunk = src // CHUNK
    ldst = dst - core * OWN
    win = ldst // 128

    # counts[k, c, w] over physical windows, then remap windows to slots
    key_w = (core * 3 + chunk) * N_WIN + win
    counts = np.bincount(key_w, minlength=NCORES * 3 * N_WIN).reshape(
        NCORES, 3, N_WIN
    )
    perm = _window_perms(counts)                    # [k, s] -> physical w
    inv_perm = np.empty_like(perm)
    np.put_along_axis(inv_perm, perm, np.arange(N_WIN)[None, :], axis=1)

    slot = inv_perm[core, win]
    key = (core * 3 + chunk) * N_WIN + slot
    counts_p = np.take_along_axis(counts, perm[:, None, :], axis=2)

    mx = np.maximum.reduce(counts_p, axis=0)        # [3, N_WIN] per slot
    seg_cap = -(-mx // SEG_ALIGN) * SEG_ALIGN

    caps = []
    for c in range(3):
        ntok = int(seg_cap[c].sum())
        caps.append(max(128, -(-ntok // 128) * 128))
    tot = int(sum(caps))

    # segment offsets + shared program structure (chunk-major, slot asc)
    base = 0
    seg_off = np.zeros((3, N_WIN), np.int64)
    structure = [None, None, None]
    scol = 0
    scol_meta = []   # (c, s, col, t0, t1) per S-column, emission order
    for c in CH_ORDER:
        off = base
        ents = []
        for s in range(N_WIN):
            if seg_cap[c, s] == 0:
                continue
            seg_off[c, s] = off
            t0, t1 = off, off + int(seg_cap[c, s])
            cols = []
            for col in range(t0 // 128, (t1 - 1) // 128 + 1):
                cols.append((scol, col))
                scol_meta.append((c, s, col, t0, t1))
                scol += 1
            ents.append((s, cols))
            off = t1
        structure[c] = ents
        base += caps[c]
    n_scol = -(-scol // 32) * 32

    first_chunk = {}
    last_chunk = {}
    for c in CH_ORDER:
        for s, _ in structure[c]:
            first_chunk.setdefault(s, c)
            last_chunk[s] = c
    for s in range(N_WIN):
        first_chunk.setdefault(s, None)

    # pre-scaled fp16 feature table, rows padded to 128 elements (256B)
    inv_sqrt = (1.0 / np.sqrt(np.asarray(degree, np.float64))).astype(np.float32)
    ftab = np.zeros((PAD_N, 128), np.float16)
    ftab[:N_NODES, :F] = (
        np.asarray(feature, np.float32) * inv_sqrt[:, None]
    ).astype(np.float16)

    Wn = np.ascontiguousarray(np.asarray(W, np.float32).astype(np.float16))
    bn = np.asarray(b, np.float32)
    b_mat = np.ascontiguousarray(np.tile(bn[None, :], (128, 1)))

    iota_host = np.ascontiguousarray(
        np.tile(np.arange(128, dtype=np.float16), (128, 1)))

    # per-core token placement: edges of (k,c,slot) go to consecutive tokens
    # at seg_off[c,slot]
    order = np.argsort(key, kind="stable")
    kstarts = np.concatenate([[0], np.cumsum(np.bincount(
        key, minlength=NCORES * 3 * N_WIN))])

    inv_sqrt_pad = np.ones(NCORES * AGG_ROWS, np.float32)
    inv_sqrt_pad.reshape(NCORES, AGG_ROWS)[:, :OWN] = inv_sqrt.reshape(
        NCORES, OWN
    )

    in_maps = []
    for k in range(NCORES):
        gv = np.zeros(tot, np.int64)
        dv = np.full(tot, -1, np.int64)
        for c in range(3):
            for s in range(N_WIN):
                if seg_cap[c, s] == 0:
                    continue
                b0 = kstarts[(k * 3 + c) * N_WIN + s]
                b1 = kstarts[(k * 3 + c) * N_WIN + s + 1]
                n = b1 - b0
                o = seg_off[c, s]
                e = order[b0:b1]
                gv[o : o + n] = src[e] - c * CHUNK
                dv[o : o + n] = ldst[e] - perm[k, s] * 128

        # S-column dst_rel stream, masked -1 outside each segment's range
        ds = np.full((n_scol, 128), -1, np.int64)
        for i, (c, s, col, t0, t1) in enumerate(scol_meta):
            lo, hi = max(t0, col * 128), min(t1, (col + 1) * 128)
            ds[i, lo - col * 128 : hi - col * 128] = dv[lo:hi]

        # rsqrt(deg) per slot: slot s holds physical window perm[k, s]
        so = inv_sqrt_pad.reshape(NCORES, N_WIN, 128)[k, perm[k]]  # [74, 128]
        so = np.ascontiguousarray(so.T)                            # [128, 74]
        in_maps.append(
            {
                "ftab": ftab,
                "gidx": _pack_idx(gv),
                "drel": np.ascontiguousarray(ds.T.astype(np.float32)),
                "s_own": so,
                "b_mat": b_mat,
                "W": Wn,
                "iota": iota_host,
            }
        )
    return structure, tuple(caps), n_scol, first_chunk, last_chunk, perm, in_maps


def _unshard(res, perm):
    """Per-core out [128, slots, 64] -> full [75000, 64]."""
    full = np.empty((N_NODES, F), np.float32)
    for k in range(NCORES):
        o = np.asarray(res[k]["out"])               # [128, N_WIN, F]
        o = o.transpose(1, 0, 2)                    # [slot, p, F]
        dest = np.empty((AGG_ROWS, F), np.float32)  # physical-window order
        dest.reshape(N_WIN, 128, F)[perm[k]] = o
        full[k * OWN : (k + 1) * OWN] = dest[:OWN]
    return full


def kernel(feature, degree, src, dst, W, b):
    structure, caps, n_scol, first_chunk, last_chunk, perm, in_maps = _prepare(
        feature, degree, src, dst, W, b
    )
    nc = _build_nc(structure, caps, n_scol, first_chunk, last_chunk)
    res = run_bass_kernel_spmd(nc, in_maps, list(range(NCORES)))
    return _unshard(res.results, perm)


# revision 13
# speedup vs baseline: 1.7255x; 1.0059x over previous
"""DegreeGCNLayer on 8 Trainium2 NeuronCores (Bass/Tile, SPMD).

h = (segment_sum(feature[src] * rsqrt(deg)[src], dst) * rsqrt(deg)) @ W + b

Sharding: nodes split 8 ways (9375/core); edges partitioned by dst owner on
the host so the segment-sum is core-local; the pre-scaled feature table is
replicated to every core (host-side replication stands in for the all-gather
of remote src features); W/b replicated.

Host prep: f~ = feature * rsqrt(degree) cast to fp16, rows padded to 128
elements (256B — the SWDGE gather descriptor minimum), split into 3 chunk
tables of <=32768 rows so gather indices fit int16.

Per-core device program:
  1. dma_gather f~[src_e] per edge (SWDGE MoE gather, 4096-edge calls; edges
     sorted by (src-chunk, dst-window) on host; 256B/edge).
  2. segment-sum on the PE: per 128-edge column build a 0/1 selection matrix
     S[j, r] = (dst_rel[j] == r) as ONE fp16 tensor_scalar(is_equal) op per
     column (per-partition scalar operand -> DVE 4x perf mode), then
     psum_win[128, 64] += S.T @ messages — exact, deterministic.
     Segments are padded to 64 tokens; a column shared by two adjacent
     windows is emitted twice with host-masked dst_rel (-1 outside the
     segment), so padding stays at 64-granularity while matmul columns
     stay 128-wide.
  3. octet flush: psum [128, <=8 windows, 64] -> SBUF agg table (DVE copy for
     the first chunk touching a window, add after).
  4. finalize per window: PE-transpose agg, cast fp16 on ACT, PE matmul
     against fp16 W, then one DVE op: out = rsqrt(deg_own) * (agg@W) + b.
  5. one partition-major output DMA [128, 74, 64]; host de-interleaves.
"""

import numpy as np

from concourse import bacc, mybir, tile
from concourse.bass_utils import run_bass_kernel_spmd
from concourse.masks import make_identity

N_NODES = 75000
N_EDGES = 1200000
F = 64
NCORES = 8
OWN = N_NODES // NCORES            # 9375
CHUNK = 32768                      # int16-indexable gather table chunk
PAD_N = 75008                      # 2*32768 + 9472
CHUNK_ROWS = (CHUNK, CHUNK, PAD_N - 2 * CHUNK)
TILE_E = 1024                      # edges per gather call; the SWDGE ring
                                   # (dynamic_dma_scratch_size/16 descriptors,
                                   # carved out of SBUF) must hold 2 calls so
                                   # desc-gen overlaps the previous transfer
GPG = TILE_E // 128                # gather columns per tile (32)
CH_ORDER = (2, 0, 1)               # chunk 2 first: window finalizes (all in
                                   # the last chunk's sweep) hide under the
                                   # big chunk-1 gather stream
SEG_ALIGN = 8                      # segment padding granularity
AGG_ROWS = 9472                    # 74 windows of 128 rows (>= OWN)
N_WIN = AGG_ROWS // 128            # 74
F32 = mybir.dt.float32
F16 = mybir.dt.float16
I16 = mybir.dt.int16


def _build_nc(structure, caps, n_scol, first_chunk, last_chunk):
    """Build the single SPMD Bass program.

    structure: per chunk, list of (window, [(scol, gcol), ...]) in stream
    order (windows ascending, gather columns ascending).
    caps: per-chunk padded token counts (multiples of 128).
    """
    nc = bacc.Bacc("TRN2", target_bir_lowering=False, debug=False,
                   num_swdge_queues=1,
                   dynamic_dma_scratch_size=32768)

    tot = sum(caps)
    ftab = nc.declare_dram_parameter("ftab", [PAD_N, 128], F16, isOutput=False)
    gidx = nc.declare_dram_parameter("gidx", [128, tot // 16], I16, isOutput=False)
    drel = nc.declare_dram_parameter("drel", [128, n_scol], F32, isOutput=False)
    s_own_in = nc.declare_dram_parameter("s_own", [128, N_WIN], F32, isOutput=False)
    b_in = nc.declare_dram_parameter("b_mat", [128, F], F32, isOutput=False)
    w_in = nc.declare_dram_parameter("W", [F, F], F16, isOutput=False)
    iota_in = nc.declare_dram_parameter("iota", [128, 128], F16, isOutput=False)
    out = nc.declare_dram_parameter("out", [128, N_WIN, F], F32, isOutput=True)

    base_tok = [0, 0, 0]
    acc_t = 0
    for c in CH_ORDER:
        base_tok[c] = acc_t
        acc_t += caps[c]

    with tile.TileContext(nc) as tc:
        with (
            tc.tile_pool(name="const", bufs=1) as constp,
            tc.tile_pool(name="idxp", bufs=1) as idxp,
            tc.tile_pool(name="aggp", bufs=1) as aggp,
        ):
            gidx_sb = idxp.tile([128, tot // 16], I16, tag="gidx")
            drel_sb = idxp.tile([128, n_scol], F32, tag="drel")
            nc.sync.dma_start(out=gidx_sb[:, :], in_=gidx[:, :])
            nc.sync.dma_start(out=drel_sb[:, :], in_=drel[:, :])

            wb = constp.tile([F, F], F16, tag="wb")
            nc.sync.dma_start(out=wb[:, :], in_=w_in[:, :])

            s_own = constp.tile([128, N_WIN], F32, tag="s_own")
            nc.sync.dma_start(out=s_own[:, :], in_=s_own_in[:, :])

            b_mat = constp.tile([128, F], F32, tag="b_mat")
            nc.sync.dma_start(out=b_mat[:, :], in_=b_in[:, :])

            ident = constp.tile([128, 128], F32, tag="ident")
            make_identity(nc, ident[:, :])

            iota_mat = constp.tile([128, 128], F16, tag="iota_mat")
            nc.sync.dma_start(out=iota_mat[:, :], in_=iota_in[:, :])

            # --- SBUF-resident agg accumulator + output buffer ----------
            agg_sb = aggp.tile([128, N_WIN, F], F32, tag="agg")
            osb_all = aggp.tile([128, N_WIN, F], F32, tag="osb_all")
            # windows never flushed by their first chunk still need zeros
            # (finalize reads agg unconditionally)
            for w in range(N_WIN):
                if first_chunk.get(w) is None:
                    nc.vector.memset(agg_sb[:, w, :], 0.0)

            with (
                tc.tile_pool(name="gath", bufs=16) as gp,
                tc.tile_pool(name="sp", bufs=12) as spool,
                tc.tile_pool(name="aps", bufs=4, space="PSUM") as apsp,
                tc.tile_pool(name="fin", bufs=3) as fp,
                tc.tile_pool(name="fps", bufs=2, space="PSUM") as fpsp,
            ):
                done_win = set()

                def finalize(w):
                    # h[m,:] = s[m] * (agg[m,:] @ W) + b   (row-scale commutes
                    # through the right-matmul)
                    done_win.add(w)
                    tp = fpsp.tile([F, 128], F32, tag="tp")
                    nc.tensor.transpose(
                        out=tp[:, :], in_=agg_sb[:, w, :], identity=ident[:, :]
                    )
                    acc = fp.tile([F, 128], F16, tag="acc")
                    nc.scalar.activation(
                        acc[:, :], tp[:, :], mybir.ActivationFunctionType.Copy
                    )
                    ot = fpsp.tile([128, F], F32, tag="ot")
                    nc.tensor.matmul(
                        out=ot[:, :], lhsT=acc[:, :], rhs=wb[:, :],
                        start=True, stop=True,
                    )
                    nc.vector.scalar_tensor_tensor(
                        out=osb_all[:, w, :], in0=ot[:, :],
                        scalar=s_own[:, w : w + 1], in1=b_mat[:, :],
                        op0=mybir.AluOpType.mult, op1=mybir.AluOpType.add,
                    )

                calls = {}      # chunk -> [(local tok0, ntok)]
                col2call = {}   # chunk -> local col -> call idx
                last_c = CH_ORDER[-1]
                for c in range(3):
                    lst = []
                    t = 0
                    while t < caps[c]:
                        n = min(TILE_E, caps[c] - t)
                        if c == last_c and t + n >= caps[c]:
                            while n > 256:
                                lst.append((t, 256))
                                t += 256
                                n -= 256
                        lst.append((t, n))
                        t += n
                    calls[c] = lst
                    m = [0] * (caps[c] // 128)
                    for ci, (t0, n) in enumerate(lst):
                        for col in range(t0 // 128, (t0 + n) // 128):
                            m[col] = ci
                    col2call[c] = m

                gdict = {}   # (chunk, call idx) -> gather tile

                def get_gt(c, ci):
                    if (c, ci) not in gdict:
                        lt0, ntok = calls[c][ci]
                        tok0 = base_tok[c] + lt0
                        gt = gp.tile([128, GPG, 128], F16, tag="gt")
                        nc.gpsimd.dma_gather(
                            gt[:, : ntok // 128, :],
                            ftab[c * CHUNK : c * CHUNK + CHUNK_ROWS[c], :],
                            gidx_sb[:, tok0 // 16 : (tok0 + ntok) // 16],
                            ntok,
                            ntok,
                            128,
                        )
                        gdict[(c, ci)] = gt
                    return gdict[(c, ci)]

                for c in CH_ORDER:
                    ents = structure[c]
                    base_col = base_tok[c] // 128
                    i = 0
                    while i < len(ents):
                        octet = [ents[i]]
                        while (len(octet) < 8 and i + len(octet) < len(ents)
                               and ents[i + len(octet)][0]
                               == octet[-1][0] + 1):
                            octet.append(ents[i + len(octet)])
                        q = len(octet)
                        w0 = octet[0][0]
                        ps = apsp.tile([128, 8, F], F32, tag="ps")
                        for j, (w, cols) in enumerate(octet):
                            for gi, (scol, gcol) in enumerate(cols):
                                ci = col2call[c][gcol - base_col]
                                gt = get_gt(c, ci)
                                lcol = gcol - base_col - calls[c][ci][0] // 128
                                st = spool.tile([128, 128], F16, tag="st")
                                nc.vector.tensor_scalar(
                                    st[:, :], iota_mat[:, :],
                                    drel_sb[:, scol : scol + 1], None,
                                    mybir.AluOpType.is_equal,
                                )
                                nc.tensor.matmul(
                                    out=ps[:, j, :],
                                    lhsT=st[:, :],
                                    rhs=gt[:, lcol, 0:F],
                                    start=(gi == 0),
                                    stop=(gi == len(cols) - 1),
                                )
                        if first_chunk[w0] == c:
                            nc.vector.tensor_copy(
                                agg_sb[:, w0 : w0 + q, :], ps[:, 0:q, :]
                            )
                        else:
                            nc.vector.tensor_tensor(
                                out=agg_sb[:, w0 : w0 + q, :],
                                in0=agg_sb[:, w0 : w0 + q, :],
                                in1=ps[:, 0:q, :],
                                op=mybir.AluOpType.add,
                            )
                        fin_run = []
                        for w, _ in octet:
                            if last_chunk[w] == c:
                                finalize(w)
                                fin_run.append(w)
                        if fin_run:
                            w_lo, w_hi = fin_run[0], fin_run[-1] + 1
                            nc.sync.dma_start(
                                out=out[:, w_lo:w_hi, :],
                                in_=osb_all[:, w_lo:w_hi, :],
                            )
                        i += q

                # windows with no edges at all: bias-only output
                for w in range(N_WIN):
                    if w not in done_win:
                        finalize(w)
                        nc.sync.dma_start(
                            out=out[:, w : w + 1, :],
                            in_=osb_all[:, w : w + 1, :],
                        )
    nc.compile()
    return nc


def _pack_idx(vals):
    """Token i -> [i%16 + 16c, i//16] for c in 0..7 (wrap-16, replicated)."""
    cols = len(vals) // 16
    a = vals.reshape(cols, 16).T
    return np.tile(a, (8, 1)).astype(np.int16)


def _window_perms(counts):
    """Per-core window->slot permutation equalizing per-slot loads across
    cores (shrinks the shared program's max-over-cores segment capacities).

    Returns perm[k, s] = physical window of core k living in slot s.
    """
    tot_kw = counts.sum(axis=1)
    perm = np.argsort(-tot_kw, axis=1)
    cnt_p = np.take_along_axis(counts, perm[:, None, :], axis=2)
    try:
        from scipy.optimize import linear_sum_assignment
    except Exception:
        return perm
    for _ in range(5):
        for k in range(NCORES):
            others = np.maximum.reduce(np.delete(cnt_p, k, axis=0), axis=0)
            ck = cnt_p[k]
            m = np.maximum(others[:, :, None], ck[:, None, :])
            cost = (-(-m // SEG_ALIGN) * SEG_ALIGN).sum(axis=0)
            _, cidx = linear_sum_assignment(cost)
            cnt_p[k] = ck[:, cidx]
            perm[k] = perm[k][cidx]
    return perm


def _prepare(feature, degree, src, dst, W, b):
    src = np.asarray(src).astype(np.int64)
    dst = np.asarray(dst).astype(np.int64)
    core = dst // OWN
    chunk = src // CHUNK
    ldst = dst - core * OWN
    win = ldst // 128

    # counts[k, c, w] over physical windows, then remap windows to slots
    key_w = (core * 3 + chunk) * N_WIN + win
    counts = np.bincount(key_w, minlength=NCORES * 3 * N_WIN).reshape(
        NCORES, 3, N_WIN
    )
    perm = _window_perms(counts)                    # [k, s] -> physical w
    inv_perm = np.empty_like(perm)
    np.put_along_axis(inv_perm, perm, np.arange(N_WIN)[None, :], axis=1)

    slot = inv_perm[core, win]
    key = (core * 3 + chunk) * N_WIN + slot
    counts_p = np.take_along_axis(counts, perm[:, None, :], axis=2)

    mx = np.maximum.reduce(counts_p, axis=0)        # [3, N_WIN] per slot
    seg_cap = -(-mx // SEG_ALIGN) * SEG_ALIGN

    caps = []
    for c in range(3):
        ntok = int(seg_cap[c].sum())
        caps.append(max(128, -(-ntok // 128) * 128))
    tot = int(sum(caps))

    # segment offsets + shared program structure (chunk-major, slot asc)
    base = 0
    seg_off = np.zeros((3, N_WIN), np.int64)
    structure = [None, None, None]
    scol = 0
    scol_meta = []   # (c, s, col, t0, t1) per S-column, emission order
    for c in CH_ORDER:
        off = base
        ents = []
        for s in range(N_WIN):
            if seg_cap[c, s] == 0:
                continue
            seg_off[c, s] = off
            t0, t1 = off, off + int(seg_cap[c, s])
            cols = []
            for col in range(t0 // 128, (t1 - 1) // 128 + 1):
                cols.append((scol, col))
                scol_meta.append((c, s, col, t0, t1))
                scol += 1
            ents.append((s, cols))
            off = t1
        structure[c] = ents
        base += caps[c]
    n_scol = -(-scol // 32) * 32

    first_chunk = {}
    last_chunk = {}
    for c in CH_ORDER:
        for s, _ in structure[c]:
            first_chunk.setdefault(s, c)
            last_chunk[s] = c
    for s in range(N_WIN):
        first_chunk.setdefault(s, None)

    # pre-scaled fp16 feature table, rows padded to 128 elements (256B)
    inv_sqrt = (1.0 / np.sqrt(np.asarray(degree, np.float64))).astype(np.float32)
    ftab = np.zeros((PAD_N, 128), np.float16)
    ftab[:N_NODES, :F] = (
        np.asarray(feature, np.float32) * inv_sqrt[:, None]
    ).astype(np.float16)

    Wn = np.ascontiguousarray(np.asarray(W, np.float32).astype(np.float16))
    bn = np.asarray(b, np.float32)
    b_mat = np.ascontiguousarray(np.tile(bn[None, :], (128, 1)))

    iota_host = np.ascontiguousarray(
        np.tile(np.arange(128, dtype=np.float16), (128, 1)))

    # per-core token placement: edges of (k,c,slot) go to consecutive tokens
    # at seg_off[c,slot]
    order = np.argsort(key, kind="stable")
    kstarts = np.concatenate([[0], np.cumsum(np.bincount(
        key, minlength=NCORES * 3 * N_WIN))])

    inv_sqrt_pad = np.ones(NCORES * AGG_ROWS, np.float32)
    inv_sqrt_pad.reshape(NCORES, AGG_ROWS)[:, :OWN] = inv_sqrt.reshape(
        NCORES, OWN
    )

    in_maps = []
    for k in range(NCORES):
        gv = np.zeros(tot, np.int64)
        dv = np.full(tot, -1, np.int64)
        for c in range(3):
            for s in range(N_WIN):
                if seg_cap[c, s] == 0:
                    continue
                b0 = kstarts[(k * 3 + c) * N_WIN + s]
                b1 = kstarts[(k * 3 + c) * N_WIN + s + 1]
                n = b1 - b0
                o = seg_off[c, s]
                e = order[b0:b1]
                gv[o : o + n] = src[e] - c * CHUNK
                dv[o : o + n] = ldst[e] - perm[k, s] * 128

        # S-column dst_rel stream, masked -1 outside each segment's range
        ds = np.full((n_scol, 128), -1, np.int64)
        for i, (c, s, col, t0, t1) in enumerate(scol_meta):
            lo, hi = max(t0, col * 128), min(t1, (col + 1) * 128)
            ds[i, lo - col * 128 : hi - col * 128] = dv[lo:hi]

        # rsqrt(deg) per slot: slot s holds physical window perm[k, s]
        so = inv_sqrt_pad.reshape(NCORES, N_WIN, 128)[k, perm[k]]  # [74, 128]
        so = np.ascontiguousarray(so.T)                            # [128, 74]
        in_maps.append(
            {
                "ftab": ftab,
                "gidx": _pack_idx(gv),
                "drel": np.ascontiguousarray(ds.T.astype(np.float32)),
                "s_own": so,
                "b_mat": b_mat,
                "W": Wn,
                "iota": iota_host,
            }
        )
    return structure, tuple(caps), n_scol, first_chunk, last_chunk, perm, in_maps


def _unshard(res, perm):
    """Per-core out [128, slots, 64] -> full [75000, 64]."""
    full = np.empty((N_NODES, F), np.float32)
    for k in range(NCORES):
        o = np.asarray(res[k]["out"])               # [128, N_WIN, F]
        o = o.transpose(1, 0, 2)                    # [slot, p, F]
        dest = np.empty((AGG_ROWS, F), np.float32)  # physical-window order
        dest.reshape(N_WIN, 128, F)[perm[k]] = o
        full[k * OWN : (k + 1) * OWN] = dest[:OWN]
    return full


def kernel(feature, degree, src, dst, W, b):
    structure, caps, n_scol, first_chunk, last_chunk, perm, in_maps = _prepare(
        feature, degree, src, dst, W, b
    )
    nc = _build_nc(structure, caps, n_scol, first_chunk, last_chunk)
    res = run_bass_kernel_spmd(nc, in_maps, list(range(NCORES)))
    return _unshard(res.results, perm)
